# revision 1
# baseline (speedup 1.0000x reference)
"""Trainium2 Bass kernel for nn_DockingTimeModel (2-layer GINE GNN + mean-pool
+ MLP head), single merged SPMD launch on 8 NeuronCores.

Sharding: data-parallel over graphs. Core c owns graphs [512c, 512(c+1)) and
their (contiguous, `batch` is sorted) node range; edges live on the core owning
their dst node. Per layer: dma_gather x[src] rows from host-compacted int16
tables -> edge linear on PE (bias folded via ones-row) -> relu(gather+lin) ->
dma_scatter_add into an HBM accumulator by local dst (dst-unique per chunk;
Tile serializes chunks so HBM read-modify-write never races) -> feat-major node
MLP on PE. Between layers, each core pre-gathers the h1 rows every peer needs
and exchanges them with one AllToAll; layer 2 gathers from the received
compact table. Mean-pool via per-tile indicator matmuls -> partials ->
one dma_gather + reduce; MLP head on-chip; output [1, 512] per core.
"""
import sys

sys.path.insert(0, "/opt/trn_rl_repo")

import math
from contextlib import ExitStack
from dataclasses import dataclass

import numpy as np

from concourse import bacc, bass, mybir, tile
from concourse import bass_utils
from concourse.masks import make_identity

F32 = mybir.dt.float32
I16 = mybir.dt.int16
AF = mybir.ActivationFunctionType
ALU = mybir.AluOpType

C = 8
P = 128
ND = 64
ED = 16
EMB = 128
USR = 12


def _wrap16(idx):
    L = len(idx)
    assert L % 16 == 0
    a = np.asarray(idx, np.int16).reshape(L // 16, 16).T
    return np.tile(a, (8, 1))


@dataclass
class Stream:
    """One layer's edge stream layout: groups of (n_chunks, chunk_size,
    table_id, kind) with kind in {"s","p"} (single / paired dst rows)."""
    groups: list          # [(k, CH, table_id, kind)]
    ESH: int = 0
    DSH: int = 0          # total didx entries

    @property
    def chunks(self):
        out = []
        off = doff = 0
        for k, CH, tb, kind in self.groups:
            nd = CH if kind == "s" else CH // 2
            for i in range(k):
                out.append((off, CH, tb, kind, doff))
                off += CH
                doff += nd
        return out


@dataclass
class CFG:
    TAB0: int
    TAB1: int
    s1: Stream
    s2: Stream
    N_SH: int
    B: int                # A2A block rows per (dst,src) pair
    NCH: int = 512
    GS: int = 512
    GSP: int = 512
    GW: int = 8
    PG: int = 2
    n_pool_idx: int = 0

    @property
    def NT(self):
        return self.N_SH // P


def _split_group(pos, dloc, split, gi):
    """Edges of table-group gi -> (paired a, paired b, singles) edge-index
    arrays. a[i], b[i] go to nodes (2m, 2m+1) for some m."""
    sel = np.nonzero((pos < split) if gi == 0 else (pos >= split))[0]
    if not len(sel):
        return (np.zeros(0, np.int64),) * 3
    d = dloc[sel]
    order = sel[np.argsort(d, kind="stable")]
    sd = dloc[order]
    pid = sd // 2
    # within each node: contiguous run; within each pair id: a-run then b-run
    a_list, b_list, s_list = [], [], []
    bounds = np.nonzero(np.diff(pid))[0] + 1
    startp = np.concatenate([[0], bounds])
    endp = np.concatenate([bounds, [len(sd)]])
    for s0, e0 in zip(startp, endp):
        seg = order[s0:e0]
        segd = sd[s0:e0]
        even = seg[segd % 2 == 0]
        odd = seg[segd % 2 == 1]
        npair = min(len(even), len(odd))
        a_list.append(even[:npair])
        b_list.append(odd[:npair])
        s_list.append(even[npair:])
        s_list.append(odd[npair:])
    cat = lambda L: (np.concatenate(L) if L else np.zeros(0, np.int64))
    return cat(a_list), cat(b_list), cat(s_list)


def _mk_stream(pos_all, dloc_all, ea_all, split, CHUNK_MAX):
    """Build per-layer stream shape: for each table group, a paired subgroup
    (dst = node-pair rows, 512B descs) and a singles subgroup."""
    Cn = len(pos_all)
    groups = []
    for gi in range(2):
        pr_n, pr_m, sg_n, sg_m = [], [1], [], [1]
        for c in range(Cn):
            a, b, sng = _split_group(pos_all[c], dloc_all[c], split, gi)
            pr_n.append(len(a))
            sg_n.append(len(sng))
            if len(a):
                pr_m.append(int(np.bincount(dloc_all[c][a] // 2).max()))
            if len(sng):
                sg_m.append(int(np.bincount(dloc_all[c][sng]).max()))
        # paired subgroup: CH counts EDGES (2 per pair); chunk mult of 256
        if max(pr_n) > 0:
            npmax = max(pr_n)  # pairs
            k = max(int(math.ceil(2 * npmax / (CHUNK_MAX * 0.95))), max(pr_m))
            CH = int(math.ceil(npmax / k * 1.08 / 128) * 256)
            CH = max(CH, 512)
            groups.append((k, CH, gi, "p"))
        if max(sg_n) > 0:
            nsmax = max(sg_n)
            k = max(int(math.ceil(nsmax / (CHUNK_MAX * 0.95))), max(sg_m))
            CH = int(math.ceil(nsmax / k * 1.15 / 128) * 128)
            CH = max(CH, 256)
            groups.append((k, CH, gi, "s"))
    st = Stream(groups=groups)
    st.ESH = sum(k * ch for k, ch, _, _ in st.groups)
    st.DSH = sum(k * (ch if kind == "s" else ch // 2)
                 for k, ch, _, kind in st.groups)
    st.DSH = int(math.ceil(st.DSH / 16) * 16)
    return st


def _assign_chunks(ids, k, CAP, retry=6):
    """Chunk assignment: for items with target ids (dst nodes or pair ids),
    id-unique per chunk via (rank_within_id + hash(id)) % k."""
    order = np.argsort(ids, kind="stable")
    si = ids[order]
    rank = np.arange(len(si)) - np.searchsorted(si, si, side="left")
    for salt in range(retry):
        cid = (rank + (si * (2654435761 + salt * 97)) % k) % k
        if np.bincount(cid, minlength=k).max() <= CAP:
            return order, cid
    raise AssertionError(
        f"chunk overflow {np.bincount(cid, minlength=k).max()} > {CAP}")


def _fill_stream(st, pos, dloc, ea, split, DUMP, retry=6):
    """Place edges into the stream. Paired chunks: pair i occupies edge slots
    (p, 2t) and (p, 2t+1) of the chunk; didx entry = pair id. Returns
    (gidx, didx, eaT)."""
    gidx = np.zeros(st.ESH, np.int16)
    didx = np.zeros(st.DSH, np.int16)
    eaT = np.zeros((ED + 1, st.ESH), np.float32)
    off = doff = 0
    for k, CH, tb, kind in st.groups:
        base = 0 if tb == 0 else split
        a, b, sng = _split_group(pos, dloc, split, tb)
        if kind == "p":
            didx[doff:doff + k * CH // 2] = DUMP // 2
            if len(a):
                pids = dloc[a] // 2
                order, cid = _assign_chunks(pids, k, CH // 2)
                for ki in range(k):
                    m = order[cid == ki]
                    npair = len(m)
                    e0 = off + ki * CH
                    d0 = doff + ki * (CH // 2)
                    # pair j -> partition j%128, blocks 2*(j//128), +1
                    j = np.arange(npair)
                    sa = e0 + (j // P) * 2 * P + (j % P)
                    sb = sa + P
                    ia, ib = a[m], b[m]
                    gidx[sa] = (pos[ia] - base).astype(np.int16)
                    gidx[sb] = (pos[ib] - base).astype(np.int16)
                    eaT[:ED, sa] = ea[ia].T
                    eaT[:ED, sb] = ea[ib].T
                    eaT[ED, sa] = 1.0
                    eaT[ED, sb] = 1.0
                    didx[d0:d0 + npair] = pids[m].astype(np.int16)
            off += k * CH
            doff += k * (CH // 2)
        else:
            didx[doff:doff + k * CH] = DUMP
            if len(sng):
                order, cid = _assign_chunks(dloc[sng], k, CH)
                for ki in range(k):
                    ke = sng[order[cid == ki]]
                    e0 = off + ki * CH
                    d0 = doff + ki * CH
                    nk = len(ke)
                    gidx[e0:e0 + nk] = (pos[ke] - base).astype(np.int16)
                    didx[d0:d0 + nk] = dloc[ke].astype(np.int16)
                    eaT[:ED, e0:e0 + nk] = ea[ke].T
                    eaT[ED, e0:e0 + nk] = 1.0
            off += k * CH
            doff += k * CH
    return gidx, didx, eaT


def _preprocess(x, edge_index, edge_attr, batch, G=4096, CHUNK_MAX=6400,
                TAB0=32768):
    src = np.asarray(edge_index[0], np.int64)
    dst = np.asarray(edge_index[1], np.int64)
    batch = np.asarray(batch, np.int64)
    ea = np.asarray(edge_attr, np.float32)
    GS = G // C
    gb = np.searchsorted(batch, np.arange(0, G + 1, GS))
    ncnt = np.diff(gb)
    NCH = 512
    N_SH = int(math.ceil(ncnt.max() / NCH) * NCH)
    owner = np.searchsorted(gb, dst, side="right") - 1

    cores = []
    for c in range(C):
        em = np.nonzero(owner == c)[0]
        s_c, d_c = src[em], dst[em]
        uniq, inv = np.unique(s_c, return_inverse=True)
        cores.append(dict(em=em, uniq=uniq, inv=inv, dloc=d_c - gb[c],
                          ea=ea[em]))
    max_m = max(len(pc["uniq"]) for pc in cores)
    assert max_m <= TAB0 + 32768
    TAB1 = int(math.ceil(max(max_m - TAB0, 128) / 128) * 128)

    # A2A block size: rows core c needs from owner o
    need = np.zeros((C, C), np.int64)
    for c in range(C):
        own = np.searchsorted(gb, cores[c]["uniq"], side="right") - 1
        cores[c]["uniq_owner"] = own
        for o in range(C):
            need[c, o] = int((own == o).sum())
    B = int(math.ceil((need.max() + 1) / 128) * 128)
    assert C * B <= TAB0 + 32768, f"A2A table too large: {C * B}"

    # L2 table position per uniq row: block(owner)*B + rank within block
    for c in range(C):
        own = cores[c]["uniq_owner"]
        r = np.zeros(len(own), np.int64)
        for o in range(C):
            m = own == o
            r[m] = np.arange(m.sum())
        cores[c]["pos2"] = (own * B + r)[cores[c]["inv"]]  # per-edge

    s1 = _mk_stream([pc["inv"] for pc in cores],
                    [pc["dloc"] for pc in cores],
                    None, TAB0, CHUNK_MAX)
    s2 = _mk_stream([pc["pos2"] for pc in cores],
                    [pc["dloc"] for pc in cores],
                    None, TAB0, CHUNK_MAX)

    GSP = max(P, int(math.ceil(GS / P) * P))
    cfg = CFG(TAB0=TAB0, TAB1=TAB1, s1=s1, s2=s2, N_SH=N_SH, B=B,
              NCH=NCH, GS=GS, GSP=GSP)

    DUMP = N_SH
    per_core = []
    for c in range(C):
        pc = cores[c]
        g1 = _fill_stream(s1, pc["inv"], pc["dloc"], pc["ea"], TAB0, DUMP)
        g2 = _fill_stream(s2, pc["pos2"], pc["dloc"], pc["ea"], TAB0, DUMP)

        n_c = ncnt[c]
        xT = np.zeros((ND, N_SH), np.float32)
        xT[:, :n_c] = np.asarray(x)[gb[c]:gb[c + 1]].T

        # a2a send-side: rows this core must send to each dest d = the local
        # node ids of x-rows dest d needs from us
        sg = np.zeros(C * B, np.int16)  # filled below (needs other cores)

        # pooling structures
        bl = batch[gb[c]:gb[c + 1]] - c * GS
        blp = np.full(N_SH, -1, np.int64)
        blp[:n_c] = bl
        NT = N_SH // P
        tiles = blp.reshape(NT, P)
        g_first = np.array([t[t >= 0].min() if (t >= 0).any() else 0
                            for t in tiles])
        relg = np.where(blp >= 0, blp - np.repeat(g_first, P), 255.0)
        cnt = np.bincount(bl, minlength=GS).astype(np.float32)
        gstart = np.searchsorted(bl, np.arange(GS))
        gend = np.searchsorted(bl, np.arange(GS), side="right")
        t_lo, t_hi = gstart // P, np.maximum(gend - 1, gstart) // P

        per_core.append(dict(
            gidx1=g1[0], didx1=g1[1], eaT1=g1[2],
            gidx2=g2[0], didx2=g2[1], eaT2=g2[2],
            xT=xT, uniq=pc["uniq"], uniq_owner=pc["uniq_owner"], n_c=n_c,
            relg=relg.astype(np.float32), g_first=g_first, cnt=cnt,
            t_lo=t_lo, t_hi=t_hi, sg=sg,
        ))

    # send-side gather indices: core o sends to dest c the rows c needs from o
    for o in range(C):
        sg = np.zeros(C * B, np.int16)
        for c in range(C):
            m = per_core[c]["uniq_owner"] == o
            rows = per_core[c]["uniq"][m] - gb[o]   # local node idx on o
            sg[c * B:c * B + len(rows)] = rows.astype(np.int16)
        per_core[o]["sg"] = sg

    cfg.GW = int(max((pc["relg"][pc["relg"] != 255.0]).max() + 1
                     if (pc["relg"] != 255.0).any() else 1 for pc in per_core))
    cfg.PG = int(max((pc["t_hi"] - pc["t_lo"] + 1)[pc["cnt"] > 0].max()
                     if (pc["cnt"] > 0).any() else 1 for pc in per_core))
    cfg.n_pool_idx = int(math.ceil(cfg.PG * cfg.GSP / 128) * 128)

    NT = cfg.NT
    ZPAD = NT * cfg.GW
    for pc in per_core:
        pidx = np.full(cfg.n_pool_idx, ZPAD, np.int16)
        for g in range(GS):
            if pc["cnt"][g] <= 0:
                continue
            for p, t in enumerate(range(pc["t_lo"][g], pc["t_hi"][g] + 1)):
                rel = g - pc["g_first"][t]
                pidx[p * cfg.GSP + g] = t * cfg.GW + rel
        pc["pool_idx"] = pidx
        pc["cnt_gm"] = np.maximum(
            np.pad(pc["cnt"], (0, cfg.GSP - GS)), 1.0
        ).reshape(cfg.GSP // P, P).T.astype(np.float32)

    relids = np.tile(np.arange(cfg.GW, dtype=np.float32), (P, 1))
    return cfg, gb, per_core, relids


def _gather_tables(cfg, per_core, x):
    out = []
    for pc in per_core:
        uniq = pc["uniq"]
        t0 = np.zeros((cfg.TAB0, ND), np.float32)
        t1 = np.zeros((cfg.TAB1, ND), np.float32)
        n0 = min(len(uniq), cfg.TAB0)
        t0[:n0] = x[uniq[:n0]]
        if len(uniq) > cfg.TAB0:
            t1[:len(uniq) - cfg.TAB0] = x[uniq[cfg.TAB0:]]
        out.append((t0, t1))
    return out


def _edge_phase(ctx, tc, nc, st, tabs, gidx_d, didx_d, eaT_d, w_e, acc_aps, tag):
    gp = ctx.enter_context(tc.tile_pool(name=f"eg{tag}", bufs=2))
    xp = ctx.enter_context(tc.tile_pool(name=f"ex{tag}", bufs=2))
    ep = ctx.enter_context(tc.tile_pool(name=f"ee{tag}", bufs=2))
    dp = ctx.enter_context(tc.tile_pool(name=f"ed{tag}", bufs=2))
    mp = ctx.enter_context(tc.tile_pool(name=f"em{tag}", bufs=2))
    pp = ctx.enter_context(tc.tile_pool(name=f"ep{tag}", bufs=2, space="PSUM"))
    for ci, (off, CH, tb, kind, doff) in enumerate(st.chunks):
        KB = CH // P
        gix = gp.tile([P, CH // 16], I16, tag="gix")
        nc.sync.dma_start(gix[:], gidx_d[:, off // 16:(off + CH) // 16])
        xg = xp.tile([P, KB * ND], F32, tag="xg")
        nc.gpsimd.dma_gather(
            out_ap=xg[:].rearrange("p (k e) -> p k e", e=ND),
            in_ap=tabs[tb], idxs_ap=gix[:],
            num_idxs=CH, num_idxs_reg=CH, elem_size=ND, single_packet=False)
        eat = ep.tile([ED + 1, CH], F32, tag="eat")
        nc.sync.dma_start(eat[:], eaT_d[:, off:off + CH])
        ND_IDX = CH if kind == "s" else CH // 2
        dix = dp.tile([P, ND_IDX // 16], I16, tag="dix")
        nc.sync.dma_start(dix[:], didx_d[:, doff // 16:(doff + ND_IDX) // 16])
        msg = mp.tile([P, KB * ND], F32, tag="msg")
        for g8 in range(0, KB, 8):
            nb = min(8, KB - g8)
            ps = pp.tile([P, 512], F32, tag="lin")
            for j in range(nb):
                b = g8 + j
                nc.tensor.matmul(out=ps[:, j * ND:(j + 1) * ND],
                                 lhsT=eat[:, b * P:(b + 1) * P],
                                 rhs=w_e[:], start=True, stop=True)
            sl = slice(g8 * ND, (g8 + nb) * ND)
            nc.vector.tensor_add(out=msg[:, sl], in0=xg[:, sl],
                                 in1=ps[:, :nb * ND])
            nc.scalar.activation(out=msg[:, sl], in_=msg[:, sl], func=AF.Relu)
        if kind == "s":
            nc.gpsimd.dma_scatter_add(
                out_ap=acc_aps[0],
                in_ap=msg[:].rearrange("p (k e) -> p k e", e=ND),
                idxs_ap=dix[:], num_idxs=CH, num_idxs_reg=CH, elem_size=ND,
                single_packet=False)
        else:
            nc.gpsimd.dma_scatter_add(
                out_ap=acc_aps[0].rearrange("(m two) e -> m (two e)", two=2),
                in_ap=msg[:].rearrange("p (k e) -> p k e", e=2 * ND),
                idxs_ap=dix[:], num_idxs=CH // 2, num_idxs_reg=CH // 2,
                elem_size=2 * ND, single_packet=False)


def _zero_dram_rows(nc, t, rows, cols, zt):
    RB = 2048
    for r0 in range(0, rows, RB):
        rb = min(RB, rows - r0)
        nc.sync.dma_start(
            out=t[r0:r0 + rb, :].rearrange("(p r) e -> p (r e)", p=P),
            in_=zt[:, :rb * cols // P])


def _node_mlp(ctx, tc, nc, cfg, accs, xT_d, ident, w1, b1, w2, b2, HID,
              outT_d, last_relu, out_sbuf_cb=None, rows_cb=None):
    NCH = cfg.NCH
    ap = ctx.enter_context(tc.tile_pool(name="np_acc", bufs=3))
    xp = ctx.enter_context(tc.tile_pool(name="np_x", bufs=2))
    hp = ctx.enter_context(tc.tile_pool(name="np_h", bufs=2))
    zp = ctx.enter_context(tc.tile_pool(name="np_z", bufs=2))
    op = ctx.enter_context(tc.tile_pool(name="np_o", bufs=2))
    tp = ctx.enter_context(tc.tile_pool(name="np_tp", bufs=2, space="PSUM"))
    mp = ctx.enter_context(tc.tile_pool(name="np_mm", bufs=1, space="PSUM"))
    rp = ctx.enter_context(tc.tile_pool(name="np_r", bufs=2))

    HID2 = w2.shape[1]
    for t in range(cfg.N_SH // NCH):
        xT = xp.tile([ND, NCH], F32)
        nc.sync.dma_start(xT[:], xT_d[:, t * NCH:(t + 1) * NCH])
        at = ap.tile([P, NCH // P * ND], F32)
        nc.sync.dma_start(
            at[:].rearrange("p (j e) -> p j e", e=ND),
            accs[0][t * NCH:(t + 1) * NCH, :].rearrange("(j p) e -> p j e", p=P))
        hT = hp.tile([ND, NCH], F32)
        for j in range(NCH // P):
            pt = tp.tile([ND, P], F32, tag="tp")
            nc.tensor.transpose(out=pt[:], in_=at[:, j * ND:(j + 1) * ND],
                                identity=ident[:])
            nc.vector.tensor_add(out=hT[:, j * P:(j + 1) * P],
                                 in0=pt[:], in1=xT[:, j * P:(j + 1) * P])
        z1p = mp.tile([HID, NCH], F32, tag="mm1")
        nc.tensor.matmul(out=z1p[:], lhsT=w1[:], rhs=hT[:], start=True, stop=True)
        z1 = zp.tile([HID, NCH], F32)
        nc.scalar.activation(out=z1[:], in_=z1p[:], func=AF.Relu, bias=b1[:])
        z2p = mp.tile([HID2, NCH], F32, tag="mm2")
        nc.tensor.matmul(out=z2p[:], lhsT=w2[:], rhs=z1[:], start=True, stop=True)
        o = op.tile([HID2, NCH], F32)
        nc.scalar.activation(out=o[:], in_=z2p[:],
                             func=AF.Relu if last_relu else AF.Identity,
                             bias=b2[:])
        if outT_d is not None:
            nc.sync.dma_start(out=outT_d[:, t * NCH:(t + 1) * NCH], in_=o[:])
        if rows_cb is not None:
            # also produce node-major rows (transpose o back)
            rt = rp.tile([P, NCH // P * HID2], F32)
            for j in range(NCH // P):
                pt2 = tp.tile([P, HID2], F32, tag="tp2")
                nc.tensor.transpose(out=pt2[:], in_=o[:, j * P:(j + 1) * P],
                                    identity=ident[:HID2, :HID2])
                nc.vector.tensor_copy(out=rt[:, j * HID2:(j + 1) * HID2],
                                      in_=pt2[:])
            rows_cb(t, rt)
        if out_sbuf_cb is not None:
            out_sbuf_cb(t, o)


def _build(cfg):
    nc = bacc.Bacc("TRN2", target_bir_lowering=False, debug=False,
                   num_devices=C)
    d = {}

    def inp(name, shape, dt=F32):
        d[name] = nc.dram_tensor(name, shape, dt, kind="ExternalInput").ap()

    inp("tab0", [cfg.TAB0, ND]); inp("tab1", [cfg.TAB1, ND])
    inp("gidx1", [P, cfg.s1.ESH // 16], I16); inp("didx1", [P, cfg.s1.DSH // 16], I16)
    inp("eaT1", [ED + 1, cfg.s1.ESH])
    inp("gidx2", [P, cfg.s2.ESH // 16], I16); inp("didx2", [P, cfg.s2.DSH // 16], I16)
    inp("eaT2", [ED + 1, cfg.s2.ESH])
    inp("xT", [ND, cfg.N_SH])
    inp("sg", [P, C * cfg.B // 16], I16)
    inp("w_e1", [ED + 1, ND]); inp("w11", [ND, ND]); inp("b11", [ND, 1])
    inp("w12", [ND, ND]); inp("b12", [ND, 1])
    inp("w_e2", [ED + 1, ND]); inp("w21", [ND, EMB]); inp("b21", [EMB, 1])
    inp("w22", [EMB, EMB]); inp("b22", [EMB, 1])
    inp("relg", [P, cfg.NT]); inp("relids", [P, cfg.GW])
    inp("pool_idx", [P, cfg.n_pool_idx // 16], I16)
    inp("cnt_gm", [P, cfg.GSP // P]); inp("usrT", [USR, cfg.GSP])
    for nm, shp in (("hw1a", [EMB, 128]), ("hw1b", [USR, 128]), ("hb1", [128, 1]),
                    ("hw2", [128, 64]), ("hb2", [64, 1]), ("hw3", [64, 32]),
                    ("hb3", [32, 1]), ("hw4", [32, 16]), ("hb4", [16, 1]),
                    ("hw5", [16, 1]), ("hb5", [1, 1])):
        inp(nm, shp)
    yT = nc.dram_tensor("yT", [1, cfg.GSP], F32, kind="ExternalOutput").ap()

    GW, PG, NT, GSP, B = cfg.GW, cfg.PG, cfg.NT, cfg.GSP, cfg.B
    NROW = NT * GW + P

    with tile.TileContext(nc) as tc, ExitStack() as ctx:
        const = ctx.enter_context(tc.tile_pool(name="const", bufs=1))

        def ld(name, shape):
            t = const.tile(shape, F32, name=f"c_{name}")
            nc.sync.dma_start(t[:], d[name])
            return t

        w_e1 = ld("w_e1", [ED + 1, ND])
        w11 = ld("w11", [ND, ND]); b11 = ld("b11", [ND, 1])
        w12 = ld("w12", [ND, ND]); b12 = ld("b12", [ND, 1])
        w_e2 = ld("w_e2", [ED + 1, ND])
        w21 = ld("w21", [ND, EMB]); b21 = ld("b21", [EMB, 1])
        w22 = ld("w22", [EMB, EMB]); b22 = ld("b22", [EMB, 1])
        relg = ld("relg", [P, cfg.NT])
        relids = ld("relids", [P, GW])
        ident = const.tile([P, P], F32, name="ident")
        make_identity(nc, ident[:])
        zt = const.tile([P, 1024], F32, name="zt")
        nc.vector.memset(zt[:], 0.0)

        dram = ctx.enter_context(tc.tile_pool(name="dram", bufs=1, space="DRAM"))
        acc1 = dram.tile([cfg.N_SH + P, ND], F32)
        acc2 = dram.tile([cfg.N_SH + P, ND], F32)
        h1T = dram.tile([ND, cfg.N_SH], F32)
        h1r = dram.tile([cfg.N_SH, ND], F32)
        a2a_in = dram.tile([C * B, ND], F32)
        a2a_out = dram.tile([C * B, ND], F32)
        parts = dram.tile([NROW, P], F32)
        _zero_dram_rows(nc, acc1, cfg.N_SH + P, ND, zt)
        _zero_dram_rows(nc, acc2, cfg.N_SH + P, ND, zt)
        nc.sync.dma_start(
            out=parts[NT * GW:NT * GW + P, :].rearrange("(p r) e -> p (r e)", p=P),
            in_=zt[:, :P])

        # ---- layer 1 edges ----
        with ExitStack() as ectx:
            _edge_phase(ectx, tc, nc, cfg.s1, (d["tab0"], d["tab1"]),
                        d["gidx1"], d["didx1"], d["eaT1"], w_e1,
                        (acc1[:],), "1")

        # ---- layer 1 nodes (h1T + h1 rows) ----
        def rows_cb(t, rt):
            nc.sync.dma_start(
                out=h1r[t * cfg.NCH:(t + 1) * cfg.NCH, :]
                .rearrange("(j p) e -> p j e", p=P),
                in_=rt[:].rearrange("p (j e) -> p j e", e=ND))

        with ExitStack() as nctx:
            _node_mlp(nctx, tc, nc, cfg, (acc1,), d["xT"], ident,
                      w11, b11, w12, b12, ND, h1T[:], last_relu=True,
                      rows_cb=rows_cb)

        # ---- exchange: pre-gather + AllToAll ----
        with ExitStack() as actx:
            agp = actx.enter_context(tc.tile_pool(name="a2a", bufs=2))
            sgp = actx.enter_context(tc.tile_pool(name="a2as", bufs=2))
            for dest in range(C):
                six = sgp.tile([P, B // 16], I16, tag="six")
                nc.sync.dma_start(six[:], d["sg"][:, dest * B // 16:(dest + 1) * B // 16])
                gt = agp.tile([P, B // P * ND], F32, tag="gt")
                nc.gpsimd.dma_gather(
                    out_ap=gt[:].rearrange("p (k e) -> p k e", e=ND),
                    in_ap=h1r[:], idxs_ap=six[:],
                    num_idxs=B, num_idxs_reg=B, elem_size=ND,
                    single_packet=False)
                nc.sync.dma_start(
                    out=a2a_in[dest * B:(dest + 1) * B, :]
                    .rearrange("(k p) e -> p k e", p=P),
                    in_=gt[:].rearrange("p (k e) -> p k e", e=ND))
            nc.gpsimd.collective_compute(
                "AllToAll", mybir.AluOpType.bypass,
                replica_groups=[list(range(C))],
                ins=[a2a_in[:].opt()], outs=[a2a_out[:].opt()])

        # ---- layer 2 edges (tables = a2a_out split at TAB0) ----
        t0hi = min(cfg.TAB0, C * B)
        t2_0 = a2a_out[:t0hi, :]
        t2_1 = a2a_out[t0hi:, :] if C * B > cfg.TAB0 else t2_0
        with ExitStack() as ectx:
            _edge_phase(ectx, tc, nc, cfg.s2, (t2_0, t2_1),
                        d["gidx2"], d["didx2"], d["eaT2"], w_e2,
                        (acc2[:],), "2")

        # ---- layer 2 nodes + pooling partials ----
        with ExitStack() as nctx:
            pool_sb = nctx.enter_context(tc.tile_pool(name="pl_sb", bufs=2))
            pool_ps = nctx.enter_context(tc.tile_pool(name="pl_ps", bufs=1, space="PSUM"))
            pool_s = nctx.enter_context(tc.tile_pool(name="pl_s", bufs=2))
            pool_nm = nctx.enter_context(tc.tile_pool(name="pl_nm", bufs=2))

            def pool_cb(t, embT):
                npart = cfg.NCH // P
                S4 = pool_s.tile([P, npart * GW], F32, tag="S4")
                nc.vector.tensor_tensor(
                    out=S4[:].rearrange("p (j g) -> p j g", g=GW),
                    in0=relg[:, t * npart:(t + 1) * npart]
                    .unsqueeze(2).broadcast_to([P, npart, GW]),
                    in1=relids[:].unsqueeze(1).broadcast_to([P, npart, GW]),
                    op=ALU.is_equal)
                for j in range(npart):
                    tl = t * npart + j
                    tps = pool_ps.tile([P, P], F32, tag="tpose")
                    nc.tensor.transpose(out=tps[:], in_=embT[:, j * P:(j + 1) * P],
                                        identity=ident[:])
                    enm = pool_nm.tile([P, P], F32)
                    nc.vector.tensor_copy(out=enm[:], in_=tps[:])
                    pps = pool_ps.tile([GW, P], F32, tag="part")
                    nc.tensor.matmul(out=pps[:], lhsT=S4[:, j * GW:(j + 1) * GW],
                                     rhs=enm[:], start=True, stop=True)
                    psb = pool_sb.tile([GW, P], F32)
                    nc.vector.tensor_copy(out=psb[:], in_=pps[:])
                    nc.sync.dma_start(out=parts[tl * GW:(tl + 1) * GW, :],
                                      in_=psb[:])

            _node_mlp(nctx, tc, nc, cfg, (acc2,), h1T[:], ident,
                      w21, b21, w22, b22, EMB, None, last_relu=False,
                      out_sbuf_cb=pool_cb)

        # ---- pool reduce + head ----
        with ExitStack() as hctx:
            hp = hctx.enter_context(tc.tile_pool(name="hd", bufs=1))
            hps = hctx.enter_context(tc.tile_pool(name="hd_ps", bufs=2, space="PSUM"))
            pix = hp.tile([P, cfg.n_pool_idx // 16], I16)
            nc.sync.dma_start(pix[:], d["pool_idx"])
            NPB = cfg.n_pool_idx // P
            gpo = hp.tile([P, NPB * P], F32)
            nc.gpsimd.dma_gather(
                out_ap=gpo[:].rearrange("p (k e) -> p k e", e=P),
                in_ap=parts[:], idxs_ap=pix[:],
                num_idxs=cfg.n_pool_idx, num_idxs_reg=cfg.n_pool_idx,
                elem_size=P, single_packet=False)
            GB = GSP // P
            v = gpo[:].rearrange("p (q b e) -> p q b e", q=PG, b=GB)
            pooled = hp.tile([P, GB * P], F32)
            pv = pooled[:].rearrange("p (b e) -> p b e", b=GB)
            if PG == 1:
                nc.vector.tensor_copy(out=pv, in_=v[:, 0])
            else:
                nc.vector.tensor_add(out=pv, in0=v[:, 0], in1=v[:, 1])
                for q in range(2, PG):
                    nc.vector.tensor_add(out=pv, in0=pv, in1=v[:, q])
            cntg = hp.tile([P, GB], F32)
            nc.sync.dma_start(cntg[:], d["cnt_gm"])
            invc = hp.tile([P, GB], F32)
            nc.vector.reciprocal(invc[:], cntg[:])
            for b in range(GB):
                nc.vector.tensor_tensor(
                    out=pooled[:, b * P:(b + 1) * P],
                    in0=pooled[:, b * P:(b + 1) * P],
                    in1=invc[:, b:b + 1].to_broadcast([P, P]), op=ALU.mult)
            embT = hp.tile([P, GSP], F32)
            for b in range(GB):
                tps = hps.tile([P, P], F32, tag="hd")
                nc.tensor.transpose(out=tps[:], in_=pooled[:, b * P:(b + 1) * P],
                                    identity=ident[:])
                nc.vector.tensor_copy(out=embT[:, b * P:(b + 1) * P], in_=tps[:])
            usrT = hp.tile([USR, GSP], F32)
            nc.sync.dma_start(usrT[:], d["usrT"])
            hw = {nm: hp.tile(d[nm].shape, F32, name=f"t_{nm}")
                  for nm in ("hw1a", "hw1b", "hb1", "hw2", "hb2", "hw3", "hb3",
                             "hw4", "hb4", "hw5", "hb5")}
            for nm, t in hw.items():
                nc.sync.dma_start(t[:], d[nm])
            z1p = hps.tile([128, GSP], F32, tag="hd")
            nc.tensor.matmul(out=z1p[:], lhsT=hw["hw1a"][:], rhs=embT[:],
                             start=True, stop=False)
            nc.tensor.matmul(out=z1p[:], lhsT=hw["hw1b"][:], rhs=usrT[:],
                             start=False, stop=True)
            z1 = hp.tile([128, GSP], F32)
            nc.scalar.activation(out=z1[:], in_=z1p[:], func=AF.Relu, bias=hw["hb1"][:])
            z2p = hps.tile([64, GSP], F32, tag="hd")
            nc.tensor.matmul(out=z2p[:], lhsT=hw["hw2"][:], rhs=z1[:], start=True, stop=True)
            z2 = hp.tile([64, GSP], F32)
            nc.scalar.activation(out=z2[:], in_=z2p[:], func=AF.Relu, bias=hw["hb2"][:])
            z3p = hps.tile([32, GSP], F32, tag="hd")
            nc.tensor.matmul(out=z3p[:], lhsT=hw["hw3"][:], rhs=z2[:], start=True, stop=True)
            z3 = hp.tile([32, GSP], F32)
            nc.scalar.activation(out=z3[:], in_=z3p[:], func=AF.Relu, bias=hw["hb3"][:])
            z4p = hps.tile([16, GSP], F32, tag="hd")
            nc.tensor.matmul(out=z4p[:], lhsT=hw["hw4"][:], rhs=z3[:], start=True, stop=True)
            z4 = hp.tile([16, GSP], F32)
            nc.scalar.activation(out=z4[:], in_=z4p[:], func=AF.Relu, bias=hw["hb4"][:])
            z5p = hps.tile([1, GSP], F32, tag="hd")
            nc.tensor.matmul(out=z5p[:], lhsT=hw["hw5"][:], rhs=z4[:], start=True, stop=True)
            z5 = hp.tile([1, GSP], F32)
            nc.scalar.activation(out=z5[:], in_=z5p[:], func=AF.Identity, bias=hw["hb5"][:])
            nc.sync.dma_start(out=yT, in_=z5[:])

    nc.compile()
    return nc


def _make_in_maps(cfg, per_core, inputs, relids):
    f32 = lambda a: np.ascontiguousarray(np.asarray(a, np.float32))
    x = f32(inputs["x"])
    usr = f32(inputs["usr"])
    tabs = _gather_tables(cfg, per_core, x)
    w_e1 = np.vstack([f32(inputs["e1_w"]), f32(inputs["e1_b"])[None, :]])
    w_e2 = np.vstack([f32(inputs["e2_w"]), f32(inputs["e2_b"])[None, :]])
    NT = cfg.NT
    in_maps = []
    for c, pc in enumerate(per_core):
        usrT = np.zeros((USR, cfg.GSP), np.float32)
        usrT[:, :cfg.GS] = usr[c * cfg.GS:(c + 1) * cfg.GS].T
        in_maps.append(dict(
            tab0=tabs[c][0], tab1=tabs[c][1],
            gidx1=_wrap16(pc["gidx1"]), didx1=_wrap16(pc["didx1"]),
            eaT1=np.ascontiguousarray(pc["eaT1"]),
            gidx2=_wrap16(pc["gidx2"]), didx2=_wrap16(pc["didx2"]),
            eaT2=np.ascontiguousarray(pc["eaT2"]),
            xT=pc["xT"], sg=_wrap16(pc["sg"]),
            w_e1=w_e1, w11=f32(inputs["n1_w1"]), b11=f32(inputs["n1_b1"])[:, None],
            w12=f32(inputs["n1_w2"]), b12=f32(inputs["n1_b2"])[:, None],
            w_e2=w_e2, w21=f32(inputs["n2_w1"]), b21=f32(inputs["n2_b1"])[:, None],
            w22=f32(inputs["n2_w2"]), b22=f32(inputs["n2_b2"])[:, None],
            relg=np.ascontiguousarray(pc["relg"].reshape(NT, P).T),
            relids=relids, pool_idx=_wrap16(pc["pool_idx"]),
            cnt_gm=pc["cnt_gm"], usrT=usrT,
            hw1a=f32(inputs["h1_w"])[:EMB], hw1b=f32(inputs["h1_w"])[EMB:],
            hb1=f32(inputs["h1_b"])[:, None],
            hw2=f32(inputs["h2_w"]), hb2=f32(inputs["h2_b"])[:, None],
            hw3=f32(inputs["h3_w"]), hb3=f32(inputs["h3_b"])[:, None],
            hw4=f32(inputs["h4_w"]), hb4=f32(inputs["h4_b"])[:, None],
            hw5=f32(inputs["h5_w"]), hb5=f32(inputs["h5_b"])[:, None]))
    return in_maps


def kernel(**inputs):
    cfg, gb, per_core, relids = _preprocess(
        np.asarray(inputs["x"], np.float32), inputs["edge_index"],
        np.asarray(inputs["edge_attr"], np.float32), inputs["batch"])
    nc = _build(cfg)
    in_maps = _make_in_maps(cfg, per_core, inputs, relids)
    res = bass_utils.run_bass_kernel_spmd(nc, in_maps, core_ids=list(range(C)))
    out = np.concatenate([res.results[c]["yT"][0, :cfg.GS] for c in range(C)])
    kernel._last = res
    return out[:, None].astype(np.float32)



# revision 2
# speedup vs baseline: 55.6907x; 55.6907x over previous
"""Trainium2 Bass kernel for nn_DockingTimeModel — dense-staged redesign v2.

Zero dynamic DMA, zero collectives. Data-parallel over graphs; per core:
  A: layer-1 GINE on own nodes — host-staged [ea|1|x_src] columns, edge
     linear matmul per 128-edge block, relu, indicator-matmul scatter into
     PSUM (edges grouped per variable-boundary dst tile: <=128 nodes and
     <=256 edges, so every tile has exactly 2 blocks), node MLP.
  B: layer-1 recompute of h1[src] for every layer-2 edge slot ("virtual
     nodes" in C's stream order), grouped per C tile; output R node-major.
  C: layer-2 GINE — edge linear, R added via one identity matmul per tile,
     relu, scatter reusing A's indicators, node MLP, mean-pool partials;
     pool-reduce + MLP head at the end.
All matmul inputs bf16 (fp32 PSUM accumulate); head fp32. Streaming DMA
batched per super-tile and issued from the Pool-engine queue.
"""
import sys

sys.path.insert(0, "/opt/trn_rl_repo")

import math
from contextlib import ExitStack

import numpy as np

from concourse import bacc, bass, mybir, tile
from concourse import bass_utils
from concourse.masks import make_identity

F32 = mybir.dt.float32
BF16 = mybir.dt.bfloat16
I16 = mybir.dt.int16
AF = mybir.ActivationFunctionType
ALU = mybir.AluOpType
F32R = mybir.dt.float32r
NPBF = mybir.dt.np(BF16)
FR = lambda ap: ap.bitcast(F32R)

C = 8
P = 128
ND = 64
ED = 16
EMB = 128
USR = 12
KEXT = ED + 1 + ND     # [ea | 1 | x_src]
KE2 = ED + 1
G = 4096
NCH = 512
ETH = 256              # max edges per tile


def _wrap16(idx):
    L = len(idx)
    assert L % 16 == 0
    a = np.asarray(idx, np.int16).reshape(L // 16, 16).T
    return np.tile(a, (8, 1))


class CFG:
    pass


def _preprocess(x, edge_index, edge_attr, batch):
    x = np.asarray(x, np.float32)
    src = np.asarray(edge_index[0], np.int64)
    dst = np.asarray(edge_index[1], np.int64)
    batch = np.asarray(batch, np.int64)
    ea = np.asarray(edge_attr, np.float32)
    N = x.shape[0]

    GS = G // C
    gb = np.searchsorted(batch, np.arange(0, G + 1, GS))
    owner = np.searchsorted(gb, dst, side="right") - 1

    order_by_dst = np.argsort(dst, kind="stable")
    indeg = np.bincount(dst, minlength=N)
    in_start = np.concatenate([[0], np.cumsum(indeg)])

    # greedy variable tile boundaries per core: <=128 nodes, <=ETH edges
    cores = []
    for c in range(C):
        n_c = int(gb[c + 1] - gb[c])
        deg = indeg[gb[c]:gb[c + 1]]
        tile_of = np.zeros(n_c, np.int64)
        off_in = np.zeros(n_c, np.int64)
        t = nodes = edges = 0
        for v in range(n_c):
            if nodes >= P or edges + deg[v] > ETH:
                t += 1; nodes = 0; edges = 0
            tile_of[v] = t
            off_in[v] = nodes
            nodes += 1; edges += int(deg[v])
        cores.append(dict(n_c=n_c, tile_of=tile_of, off_in=off_in,
                          ntile=t + 1))

    NT = int(math.ceil(max(pc["ntile"] for pc in cores) / 4) * 4)
    N_SH = NT * P
    NST = NT // 4
    KA = ETH // P                      # blocks per tile (exactly 2)
    NBLK = NT * KA
    ESH_A = NBLK * P
    bo = np.arange(NT + 1) * KA

    # per-core edge -> slot
    for c, pc in enumerate(cores):
        em = np.nonzero(owner == c)[0]
        dloc = dst[em] - gb[c]
        t_of = pc["tile_of"][dloc]
        o = np.argsort(t_of, kind="stable")
        em, dloc, t_of = em[o], dloc[o], t_of[o]
        cnt_t = np.bincount(t_of, minlength=NT)
        assert cnt_t.max() <= ETH
        st_off = np.concatenate([[0], np.cumsum(cnt_t)])
        rank = np.arange(len(em)) - st_off[t_of]
        slot = t_of * ETH + rank
        pc.update(em=em, dloc=dloc, slot=slot)
        vsrc = np.full(ESH_A, -1, np.int64)
        vsrc[slot] = src[em]
        pc["vsrc"] = vsrc
        # node positions
        pos = pc["tile_of"] * P + pc["off_in"]
        pc["pos"] = pos

    # B group (= A/C block) edge counts
    kB = np.ones(NBLK, np.int64)
    for pc in cores:
        vs = pc["vsrc"]
        deg = np.where(vs >= 0, indeg[np.maximum(vs, 0)], 0)
        gcnt = deg.reshape(NBLK, P).sum(1)
        kB = np.maximum(kB, (gcnt + P - 1) // P)
    boB = np.concatenate([[0], np.cumsum(kB)])
    NBLKB = int(boB[-1])
    ESH_B = NBLKB * P

    cfg = CFG()
    cfg.N_SH, cfg.NT, cfg.NST, cfg.KA = N_SH, NT, NST, KA
    cfg.bo, cfg.NBLK, cfg.ESH_A = bo, NBLK, ESH_A
    cfg.kB, cfg.boB, cfg.NBLKB, cfg.ESH_B = kB, boB, NBLKB, ESH_B
    cfg.GS = GS
    cfg.GSP = max(P, int(math.ceil(GS / P) * P))
    assert kB.max() <= 8, kB.max()

    per_core = []
    for c, pc in enumerate(cores):
        em, slot, vsrc = pc["em"], pc["slot"], pc["vsrc"]
        n_c, pos = pc["n_c"], pc["pos"]
        eaExtA = np.zeros((KEXT, ESH_A), np.float32)
        eaExtA[:ED, slot] = ea[em].T
        eaExtA[ED, slot] = 1.0
        eaExtA[ED + 1:, slot] = x[src[em]].T
        dstrelA = np.full(ESH_A, 255.0, np.float32)
        dstrelA[slot] = pc["off_in"][pc["dloc"]].astype(np.float32)
        eaT2C = np.zeros((KE2, ESH_A), np.float32)
        eaT2C[:ED, slot] = ea[em].T
        eaT2C[ED, slot] = 1.0
        eaExtB = np.zeros((KEXT, ESH_B), np.float32)
        vrelB = np.full(ESH_B, 255.0, np.float32)
        xvT = np.zeros((ND, ESH_A), np.float32)
        real = vsrc >= 0
        xvT[:, real] = x[vsrc[real]].T
        deg = np.where(real, indeg[np.maximum(vsrc, 0)], 0)
        for g in range(NBLK):
            vv = vsrc[g * P:(g + 1) * P]
            dd = deg[g * P:(g + 1) * P]
            tot = int(dd.sum())
            if tot == 0:
                continue
            starts = in_start[np.maximum(vv, 0)]
            reps = np.repeat(starts, dd) + (
                np.arange(tot) - np.repeat(np.concatenate([[0], np.cumsum(dd)])[:-1], dd))
            eids = order_by_dst[reps]
            ppos = boB[g] * P + np.arange(tot)
            eaExtB[:ED, ppos] = ea[eids].T
            eaExtB[ED, ppos] = 1.0
            eaExtB[ED + 1:, ppos] = x[src[eids]].T
            vrelB[ppos] = np.repeat(np.arange(P), dd).astype(np.float32)

        # pooling structures on positions
        bl = batch[gb[c]:gb[c + 1]] - c * GS
        blp = np.full(N_SH, -1, np.int64)
        blp[pos] = bl
        tiles = blp.reshape(NT, P)
        g_first = np.array([t[t >= 0].min() if (t >= 0).any() else 0
                            for t in tiles])
        relg = np.where(blp >= 0, blp - np.repeat(g_first, P), 255.0)
        cnt = np.bincount(bl, minlength=GS).astype(np.float32)
        gstart = np.searchsorted(bl, np.arange(GS))
        gend = np.searchsorted(bl, np.arange(GS), side="right")
        t_lo = pc["tile_of"][np.minimum(gstart, n_c - 1)]
        t_hi = pc["tile_of"][np.maximum(gend - 1, gstart)]

        xT = np.zeros((ND, N_SH), np.float32)
        xT[:, pos] = x[gb[c]:gb[c + 1]].T

        per_core.append(dict(
            eaExtA=eaExtA, dstrelA=dstrelA.reshape(NBLK, P).T,
            eaT2C=eaT2C, eaExtB=eaExtB, vrelB=vrelB.reshape(NBLKB, P).T,
            xvT=xvT, n_c=n_c, xT=xT, pos=pos,
            relg=relg.astype(np.float32), g_first=g_first, cnt=cnt,
            t_lo=t_lo, t_hi=t_hi,
        ))

    cfg.GW = int(max((pc["relg"][pc["relg"] != 255.0]).max() + 1
                     if (pc["relg"] != 255.0).any() else 1 for pc in per_core))
    cfg.PG = int(max((pc["t_hi"] - pc["t_lo"] + 1)[pc["cnt"] > 0].max()
                     if (pc["cnt"] > 0).any() else 1 for pc in per_core))
    cfg.n_pool_idx = int(math.ceil(cfg.PG * cfg.GSP / P) * P)

    ZPAD = NT * cfg.GW
    for pc in per_core:
        pidx = np.full(cfg.n_pool_idx, ZPAD, np.int16)
        for g in range(GS):
            if pc["cnt"][g] <= 0:
                continue
            for p_, t in enumerate(range(pc["t_lo"][g], pc["t_hi"][g] + 1)):
                rel = g - pc["g_first"][t]
                pidx[p_ * cfg.GSP + g] = t * cfg.GW + rel
        pc["pool_idx"] = pidx
        pc["cnt_gm"] = np.maximum(
            np.pad(pc["cnt"], (0, cfg.GSP - GS)), 1.0
        ).reshape(cfg.GSP // P, P).T.astype(np.float32)

    relids = np.tile(np.arange(cfg.GW, dtype=np.float32), (P, 1))
    return cfg, gb, per_core, relids


def _build(cfg):
    nc = bacc.Bacc("TRN2", target_bir_lowering=False, debug=False,
                   num_devices=C)
    d = {}

    def inp(name, shape, dt=F32):
        d[name] = nc.dram_tensor(name, shape, dt, kind="ExternalInput").ap()

    NBLK, NBLKB, NT, NST, KA = cfg.NBLK, cfg.NBLKB, cfg.NT, cfg.NST, cfg.KA
    bo, kB, boB = cfg.bo, cfg.kB, cfg.boB
    GW, PG, GSP = cfg.GW, cfg.PG, cfg.GSP
    NROW = NT * GW + P

    inp("eaExtA", [KEXT, cfg.ESH_A], BF16)
    inp("eaExtB", [KEXT, cfg.ESH_B], BF16)
    inp("eaT2C", [KE2, cfg.ESH_A], BF16)
    inp("dstrelA", [P, NBLK], BF16)
    inp("vrelB", [P, NBLKB], BF16)
    inp("xvT", [ND, cfg.ESH_A])
    inp("xT", [ND, cfg.N_SH])
    inp("iota", [P, P], BF16)
    inp("identb", [P, P], BF16)
    inp("W1e", [KEXT, ND], BF16)
    inp("W2e", [KE2, ND], BF16)
    inp("w11", [ND, ND], F32R)
    inp("b11", [ND, 1])
    inp("w12", [ND, ND], F32R)
    inp("b12", [ND, 1])
    inp("b12rep", [1, ETH // P * ND], F32R)
    inp("ones1", [1, P], F32R)
    inp("w21", [ND, EMB], F32R)
    inp("b21", [EMB, 1])
    inp("w22", [EMB, EMB], F32R)
    inp("b22", [EMB, 1])
    inp("relg", [P, NT], BF16)
    inp("relids", [P, GW], BF16)
    inp("pool_idx", [P, cfg.n_pool_idx // 16], I16)
    inp("cnt_gm", [P, GSP // P])
    inp("usrT", [USR, GSP])
    for nm, shp in (("hw1a", [EMB, 128]), ("hw1b", [USR, 128]), ("hb1", [128, 1]),
                    ("hw2", [128, 64]), ("hb2", [64, 1]), ("hw3", [64, 32]),
                    ("hb3", [32, 1]), ("hw4", [32, 16]), ("hb4", [16, 1]),
                    ("hw5", [16, 1]), ("hb5", [1, 1])):
        inp(nm, shp)
    yT = nc.dram_tensor("yT", [1, GSP], F32, kind="ExternalOutput").ap()

    with tile.TileContext(nc) as tc, ExitStack() as ctx:
        const = ctx.enter_context(tc.tile_pool(name="const", bufs=1))

        def ld(name, shape, dt=F32):
            t = const.tile(shape, dt, name=f"c_{name}")
            nc.sync.dma_start(t[:], d[name])
            return t

        W1e = ld("W1e", [KEXT, ND], BF16)
        W2e = ld("W2e", [KE2, ND], BF16)
        w11 = ld("w11", [ND, ND], F32R); b11 = ld("b11", [ND, 1])
        w12 = ld("w12", [ND, ND], F32R); b12 = ld("b12", [ND, 1])
        b12rep = ld("b12rep", [1, ETH // P * ND], F32R)
        ones1 = ld("ones1", [1, P], F32R)
        w21 = ld("w21", [ND, EMB], F32R); b21 = ld("b21", [EMB, 1])
        w22 = ld("w22", [EMB, EMB], F32R); b22 = ld("b22", [EMB, 1])
        iota = ld("iota", [P, P], BF16)
        identb = ld("identb", [P, P], BF16)
        dstrelA = ld("dstrelA", [P, NBLK], BF16)
        vrelB = ld("vrelB", [P, NBLKB], BF16)
        xT = ld("xT", [ND, cfg.N_SH])
        relg = ld("relg", [P, NT], BF16)
        relids = ld("relids", [P, GW], BF16)
        ident = const.tile([P, P], F32, name="ident")
        make_identity(nc, ident[:])
        zt = const.tile([P, P], F32, name="zt")
        nc.vector.memset(zt[:], 0.0)

        dram = ctx.enter_context(tc.tile_pool(name="dram", bufs=1, space="DRAM"))
        parts = dram.tile([NROW, P], F32)
        nc.sync.dma_start(
            out=parts[NT * GW:NT * GW + P, :].rearrange("(p r) e -> p (r e)", p=P),
            in_=zt[:, :P])

        mctx = ctx.enter_context(ExitStack())
        psE = mctx.enter_context(tc.tile_pool(name="psE", bufs=3, space="PSUM"))
        psA = mctx.enter_context(tc.tile_pool(name="psA", bufs=3, space="PSUM"))
        psZ = mctx.enter_context(tc.tile_pool(name="psZ", bufs=2, space="PSUM"))
        eaAp = mctx.enter_context(tc.tile_pool(name="eaAp", bufs=2))
        eaBp = mctx.enter_context(tc.tile_pool(name="eaBp", bufs=2))
        ea2p = mctx.enter_context(tc.tile_pool(name="ea2p", bufs=2))
        xvp = mctx.enter_context(tc.tile_pool(name="xvp", bufs=2))
        msgp = mctx.enter_context(tc.tile_pool(name="msgp", bufs=3))
        indp = mctx.enter_context(tc.tile_pool(name="indp", bufs=12))
        hvp = mctx.enter_context(tc.tile_pool(name="hvp", bufs=3))
        z1p = mctx.enter_context(tc.tile_pool(name="z1p", bufs=3))
        Rp = mctx.enter_context(tc.tile_pool(name="Rp", bufs=10))
        hp = mctx.enter_context(tc.tile_pool(name="hp", bufs=2))
        h1p = mctx.enter_context(tc.tile_pool(name="h1p", bufs=2))
        embp = mctx.enter_context(tc.tile_pool(name="embp", bufs=2))
        s4p = mctx.enter_context(tc.tile_pool(name="s4p", bufs=2))
        psbp = mctx.enter_context(tc.tile_pool(name="psbp", bufs=2))

        for st in range(NST):
            t0, t1 = 4 * st, 4 * st + 4
            sA0, sA1 = bo[t0] * P, bo[t1] * P         # A/C slot range
            sB0, sB1 = boB[bo[t0]] * P, boB[bo[t1]] * P
            # ---- batched streaming loads for this super-tile ----
            eaA = eaAp.tile([KEXT, sA1 - sA0], BF16, tag="eaA")
            nc.gpsimd.dma_start(eaA[:], d["eaExtA"][:, sA0:sA1])
            eaB = eaBp.tile([KEXT, sB1 - sB0], BF16, tag="eaB")
            nc.gpsimd.dma_start(eaB[:], d["eaExtB"][:, sB0:sB1])
            ea2 = ea2p.tile([KE2, sA1 - sA0], BF16, tag="ea2")
            nc.gpsimd.dma_start(ea2[:], d["eaT2C"][:, sA0:sA1])
            xvt = xvp.tile([ND, sA1 - sA0], F32, tag="xvt")
            nc.gpsimd.dma_start(xvt[:], d["xvT"][:, sA0:sA1])

            R_of = {}
            ind_of = {}
            # ---- B: recompute R rows, one chunk per tile ----
            for t in range(t0, t1):
                L = KA * P
                co = (bo[t] - bo[t0]) * P              # chunk offset in xvt
                aggB = psA.tile([ND, L], F32, tag="agg")
                for gj in range(KA):
                    g = bo[t] + gj
                    kb = int(kB[g])
                    eL = psE.tile([P, kb * ND], F32, tag="eL")
                    msgB = msgp.tile([P, kb * ND], BF16, tag="msgB")
                    eoff = (boB[g] - boB[bo[t0]]) * P
                    for b in range(kb):
                        nc.tensor.matmul(
                            out=eL[:, b * ND:(b + 1) * ND],
                            lhsT=eaB[:, eoff + b * P:eoff + (b + 1) * P],
                            rhs=W1e[:], start=True, stop=True)
                    nc.scalar.activation(out=msgB[:], in_=eL[:], func=AF.Relu)
                    indB = indp.tile([P, kb * P], BF16, tag="indB")
                    nc.vector.tensor_tensor(
                        out=indB[:].rearrange("p (k e) -> p k e", e=P),
                        in0=vrelB[:, boB[g]:boB[g] + kb]
                        .unsqueeze(2).broadcast_to([P, kb, P]),
                        in1=iota[:].unsqueeze(1).broadcast_to([P, kb, P]),
                        op=ALU.is_equal)
                    for b in range(kb):
                        nc.tensor.matmul(out=aggB[:, gj * P:(gj + 1) * P],
                                         lhsT=msgB[:, b * ND:(b + 1) * ND],
                                         rhs=indB[:, b * P:(b + 1) * P],
                                         start=(b == 0), stop=(b == kb - 1))
                hv = hvp.tile([ND, L], F32R, tag="hv")
                nc.vector.tensor_add(out=hv[:], in0=aggB[:],
                                     in1=xvt[:, co:co + L])
                z1ps = psZ.tile([ND, L], F32, tag="z")
                nc.tensor.matmul(out=z1ps[:], lhsT=w11[:], rhs=hv[:],
                                 start=True, stop=True)
                z1sb = z1p.tile([ND, L], F32R, tag="z1B")
                nc.scalar.activation(out=z1sb[:], in_=z1ps[:], func=AF.Relu,
                                     bias=b11[:])
                Rps = psZ.tile([P, KA * ND], F32, tag="z")
                for gj in range(KA):
                    nc.tensor.matmul(out=Rps[:, gj * ND:(gj + 1) * ND],
                                     lhsT=z1sb[:, gj * P:(gj + 1) * P],
                                     rhs=w12[:], start=True, stop=False)
                    nc.tensor.matmul(out=Rps[:, gj * ND:(gj + 1) * ND],
                                     lhsT=ones1[:], rhs=b12rep[:, :ND],
                                     start=False, stop=True)
                Rt = Rp.tile([P, KA * ND], BF16, tag="R")
                nc.scalar.activation(out=Rt[:], in_=Rps[:], func=AF.Relu)
                R_of[t] = Rt
            # ---- A ----
            agg1 = psA.tile([ND, NCH], F32, tag="agg")
            for j, t in enumerate(range(t0, t1)):
                co = (bo[t] - bo[t0]) * P
                eL = psE.tile([P, KA * ND], F32, tag="eL")
                msgA = msgp.tile([P, KA * ND], BF16, tag="msgA")
                for b in range(KA):
                    nc.tensor.matmul(
                        out=eL[:, b * ND:(b + 1) * ND],
                        lhsT=eaA[:, co + b * P:co + (b + 1) * P],
                        rhs=W1e[:], start=True, stop=True)
                nc.scalar.activation(out=msgA[:], in_=eL[:], func=AF.Relu)
                indA = indp.tile([P, KA * P], BF16, tag="indA")
                nc.vector.tensor_tensor(
                    out=indA[:].rearrange("p (k e) -> p k e", e=P),
                    in0=dstrelA[:, bo[t]:bo[t] + KA]
                    .unsqueeze(2).broadcast_to([P, KA, P]),
                    in1=iota[:].unsqueeze(1).broadcast_to([P, KA, P]),
                    op=ALU.is_equal)
                ind_of[t] = indA
                for b in range(KA):
                    nc.tensor.matmul(out=agg1[:, j * P:(j + 1) * P],
                                     lhsT=msgA[:, b * ND:(b + 1) * ND],
                                     rhs=indA[:, b * P:(b + 1) * P],
                                     start=(b == 0), stop=(b == KA - 1))
            hT = hp.tile([ND, NCH], F32R, tag="hA")
            nc.vector.tensor_add(out=hT[:], in0=agg1[:],
                                 in1=xT[:, st * NCH:(st + 1) * NCH])
            z1ps = psZ.tile([ND, NCH], F32, tag="z")
            nc.tensor.matmul(out=z1ps[:], lhsT=w11[:], rhs=hT[:],
                             start=True, stop=True)
            z1sb = z1p.tile([ND, NCH], F32R, tag="z1A")
            nc.scalar.activation(out=z1sb[:], in_=z1ps[:], func=AF.Relu,
                                 bias=b11[:])
            h1ps = psZ.tile([ND, NCH], F32, tag="z")
            nc.tensor.matmul(out=h1ps[:], lhsT=w12[:], rhs=z1sb[:],
                             start=True, stop=True)
            h1T = h1p.tile([ND, NCH], F32, tag="h1T")
            nc.scalar.activation(out=h1T[:], in_=h1ps[:], func=AF.Relu,
                                 bias=b12[:])
            # ---- C ----
            agg2 = psA.tile([ND, NCH], F32, tag="agg")
            for j, t in enumerate(range(t0, t1)):
                co = (bo[t] - bo[t0]) * P
                eL2 = psE.tile([P, KA * ND], F32, tag="eL")
                for b in range(KA):
                    nc.tensor.matmul(
                        out=eL2[:, b * ND:(b + 1) * ND],
                        lhsT=ea2[:, co + b * P:co + (b + 1) * P],
                        rhs=W2e[:], start=True, stop=False)
                    nc.tensor.matmul(
                        out=eL2[:, b * ND:(b + 1) * ND],
                        lhsT=identb[:], rhs=R_of[t][:, b * ND:(b + 1) * ND],
                        start=False, stop=True)
                msg2 = msgp.tile([P, KA * ND], BF16, tag="msg2")
                nc.scalar.activation(out=msg2[:], in_=eL2[:], func=AF.Relu)
                for b in range(KA):
                    nc.tensor.matmul(out=agg2[:, j * P:(j + 1) * P],
                                     lhsT=msg2[:, b * ND:(b + 1) * ND],
                                     rhs=ind_of[t][:, b * P:(b + 1) * P],
                                     start=(b == 0), stop=(b == KA - 1))
            hT2 = hp.tile([ND, NCH], F32R, tag="hC")
            nc.vector.tensor_add(out=hT2[:], in0=agg2[:], in1=h1T[:])
            z1ps2 = psZ.tile([EMB, NCH], F32, tag="z")
            nc.tensor.matmul(out=z1ps2[:], lhsT=w21[:], rhs=hT2[:],
                             start=True, stop=True)
            z1Csb = z1p.tile([EMB, NCH], F32R, tag="z1C")
            nc.scalar.activation(out=z1Csb[:], in_=z1ps2[:], func=AF.Relu,
                                 bias=b21[:])
            z2ps = psZ.tile([P, 4 * EMB], F32, tag="z")
            for j in range(4):
                nc.tensor.matmul(out=z2ps[:, j * EMB:(j + 1) * EMB],
                                 lhsT=z1Csb[:, j * P:(j + 1) * P],
                                 rhs=w22[:], start=True, stop=True)
            emb_nm = embp.tile([P, 4 * EMB], BF16, tag="emb")
            nc.scalar.activation(out=emb_nm[:], in_=z2ps[:], func=AF.Identity)
            S4 = s4p.tile([P, 4 * GW], BF16, tag="S4")
            nc.vector.tensor_tensor(
                out=S4[:].rearrange("p (k g) -> p k g", g=GW),
                in0=relg[:, t0:t1].unsqueeze(2).broadcast_to([P, 4, GW]),
                in1=relids[:].unsqueeze(1).broadcast_to([P, 4, GW]),
                op=ALU.is_equal)
            psb = psbp.tile([GW, 4 * P], F32, tag="psb")
            for j in range(4):
                pps = psZ.tile([GW, P], F32, tag="z")
                nc.tensor.matmul(out=pps[:], lhsT=S4[:, j * GW:(j + 1) * GW],
                                 rhs=emb_nm[:, j * EMB:(j + 1) * EMB],
                                 start=True, stop=True)
                nc.vector.tensor_copy(out=psb[:, j * P:(j + 1) * P], in_=pps[:])
            nc.gpsimd.dma_start(
                out=parts[t0 * GW:t1 * GW, :].rearrange("(k g) e -> g k e", g=GW),
                in_=psb[:].rearrange("g (k e) -> g k e", e=P))

        # ---- pool reduce + head ----
        mctx.close()
        with ExitStack() as hctx:
            hpool = hctx.enter_context(tc.tile_pool(name="hd", bufs=1))
            hps = hctx.enter_context(tc.tile_pool(name="hd_ps", bufs=2, space="PSUM"))
            pix = hpool.tile([P, cfg.n_pool_idx // 16], I16)
            nc.sync.dma_start(pix[:], d["pool_idx"])
            NPB = cfg.n_pool_idx // P
            gpo = hpool.tile([P, NPB * P], F32)
            nc.gpsimd.dma_gather(
                out_ap=gpo[:].rearrange("p (k e) -> p k e", e=P),
                in_ap=parts[:], idxs_ap=pix[:],
                num_idxs=cfg.n_pool_idx, num_idxs_reg=cfg.n_pool_idx,
                elem_size=P, single_packet=False)
            GB = GSP // P
            v = gpo[:].rearrange("p (q b e) -> p q b e", q=PG, b=GB)
            pooled = hpool.tile([P, GB * P], F32)
            pv = pooled[:].rearrange("p (b e) -> p b e", b=GB)
            if PG == 1:
                nc.vector.tensor_copy(out=pv, in_=v[:, 0])
            else:
                nc.vector.tensor_add(out=pv, in0=v[:, 0], in1=v[:, 1])
                for q in range(2, PG):
                    nc.vector.tensor_add(out=pv, in0=pv, in1=v[:, q])
            cntg = hpool.tile([P, GB], F32)
            nc.sync.dma_start(cntg[:], d["cnt_gm"])
            invc = hpool.tile([P, GB], F32)
            nc.vector.reciprocal(invc[:], cntg[:])
            for b in range(GB):
                nc.vector.tensor_tensor(
                    out=pooled[:, b * P:(b + 1) * P],
                    in0=pooled[:, b * P:(b + 1) * P],
                    in1=invc[:, b:b + 1].to_broadcast([P, P]), op=ALU.mult)
            embT = hpool.tile([P, GSP], F32)
            for b in range(GB):
                tps = hps.tile([P, P], F32, tag="hd")
                nc.tensor.transpose(out=tps[:], in_=pooled[:, b * P:(b + 1) * P],
                                    identity=ident[:])
                nc.vector.tensor_copy(out=embT[:, b * P:(b + 1) * P], in_=tps[:])
            nc.scalar.activation(out=embT[:], in_=embT[:], func=AF.Identity,
                                 bias=b22[:])
            usrT = hpool.tile([USR, GSP], F32)
            nc.sync.dma_start(usrT[:], d["usrT"])
            hw = {nm: hpool.tile(d[nm].shape, F32, name=f"t_{nm}")
                  for nm in ("hw1a", "hw1b", "hb1", "hw2", "hb2", "hw3", "hb3",
                             "hw4", "hb4", "hw5", "hb5")}
            for nm, t in hw.items():
                nc.sync.dma_start(t[:], d[nm])
            z1h = hps.tile([128, GSP], F32, tag="hd")
            nc.tensor.matmul(out=z1h[:], lhsT=hw["hw1a"][:], rhs=embT[:],
                             start=True, stop=False)
            nc.tensor.matmul(out=z1h[:], lhsT=hw["hw1b"][:], rhs=usrT[:],
                             start=False, stop=True)
            z1s = hpool.tile([128, GSP], F32)
            nc.scalar.activation(out=z1s[:], in_=z1h[:], func=AF.Relu, bias=hw["hb1"][:])
            z2h = hps.tile([64, GSP], F32, tag="hd")
            nc.tensor.matmul(out=z2h[:], lhsT=hw["hw2"][:], rhs=z1s[:], start=True, stop=True)
            z2s = hpool.tile([64, GSP], F32)
            nc.scalar.activation(out=z2s[:], in_=z2h[:], func=AF.Relu, bias=hw["hb2"][:])
            z3h = hps.tile([32, GSP], F32, tag="hd")
            nc.tensor.matmul(out=z3h[:], lhsT=hw["hw3"][:], rhs=z2s[:], start=True, stop=True)
            z3s = hpool.tile([32, GSP], F32)
            nc.scalar.activation(out=z3s[:], in_=z3h[:], func=AF.Relu, bias=hw["hb3"][:])
            z4h = hps.tile([16, GSP], F32, tag="hd")
            nc.tensor.matmul(out=z4h[:], lhsT=hw["hw4"][:], rhs=z3s[:], start=True, stop=True)
            z4s = hpool.tile([16, GSP], F32)
            nc.scalar.activation(out=z4s[:], in_=z4h[:], func=AF.Relu, bias=hw["hb4"][:])
            z5h = hps.tile([1, GSP], F32, tag="hd")
            nc.tensor.matmul(out=z5h[:], lhsT=hw["hw5"][:], rhs=z4s[:], start=True, stop=True)
            z5s = hpool.tile([1, GSP], F32)
            nc.scalar.activation(out=z5s[:], in_=z5h[:], func=AF.Identity, bias=hw["hb5"][:])
            nc.sync.dma_start(out=yT, in_=z5s[:])

    nc.compile()
    return nc


def _make_in_maps(cfg, gb, per_core, relids, inputs):
    f32 = lambda a: np.ascontiguousarray(np.asarray(a, np.float32))
    bf = lambda a: np.ascontiguousarray(np.asarray(a, np.float32)).astype(NPBF)
    W1ext = np.vstack([f32(inputs["e1_w"]), f32(inputs["e1_b"])[None, :],
                       np.eye(ND, dtype=np.float32)])
    W2ext = np.vstack([f32(inputs["e2_w"]), f32(inputs["e2_b"])[None, :]])
    usr = f32(inputs["usr"])
    iota = np.tile(np.arange(P, dtype=np.float32), (P, 1))
    identb = np.eye(P, dtype=np.float32)
    b12rep = np.tile(f32(inputs["n1_b2"]), ETH // P)[None, :]
    in_maps = []
    for c, pc in enumerate(per_core):
        usrT = np.zeros((USR, cfg.GSP), np.float32)
        usrT[:, :cfg.GS] = usr[c * cfg.GS:(c + 1) * cfg.GS].T
        in_maps.append(dict(
            eaExtA=bf(pc["eaExtA"]), eaExtB=bf(pc["eaExtB"]),
            eaT2C=bf(pc["eaT2C"]), dstrelA=bf(pc["dstrelA"]),
            vrelB=bf(pc["vrelB"]), xvT=f32(pc["xvT"]),
            xT=f32(pc["xT"]), iota=bf(iota), identb=bf(identb),
            W1e=bf(W1ext), W2e=bf(W2ext),
            w11=f32(inputs["n1_w1"]), b11=f32(inputs["n1_b1"])[:, None],
            w12=f32(inputs["n1_w2"]), b12=f32(inputs["n1_b2"])[:, None],
            b12rep=f32(b12rep), ones1=np.ones((1, P), np.float32),
            w21=f32(inputs["n2_w1"]), b21=f32(inputs["n2_b1"])[:, None],
            w22=f32(inputs["n2_w2"]), b22=f32(inputs["n2_b2"])[:, None],
            relg=bf(pc["relg"].reshape(cfg.NT, P).T), relids=bf(relids),
            pool_idx=_wrap16(pc["pool_idx"]), cnt_gm=pc["cnt_gm"], usrT=usrT,
            hw1a=f32(inputs["h1_w"])[:EMB], hw1b=f32(inputs["h1_w"])[EMB:],
            hb1=f32(inputs["h1_b"])[:, None],
            hw2=f32(inputs["h2_w"]), hb2=f32(inputs["h2_b"])[:, None],
            hw3=f32(inputs["h3_w"]), hb3=f32(inputs["h3_b"])[:, None],
            hw4=f32(inputs["h4_w"]), hb4=f32(inputs["h4_b"])[:, None],
            hw5=f32(inputs["h5_w"]), hb5=f32(inputs["h5_b"])[:, None]))
    return in_maps


def kernel(**inputs):
    cfg, gb, per_core, relids = _preprocess(
        np.asarray(inputs["x"], np.float32), inputs["edge_index"],
        np.asarray(inputs["edge_attr"], np.float32), inputs["batch"])
    nc = _build(cfg)
    in_maps = _make_in_maps(cfg, gb, per_core, relids, inputs)
    res = bass_utils.run_bass_kernel_spmd(nc, in_maps, core_ids=list(range(C)))
    out = np.concatenate([res.results[c]["yT"][0, :cfg.GS] for c in range(C)])
    kernel._last = res
    return out[:, None].astype(np.float32)


# revision 3
# speedup vs baseline: 65.8480x; 1.1824x over previous
"""Trainium2 Bass kernel for nn_DockingTimeModel — dense-staged redesign v2.

Zero dynamic DMA, zero collectives. Data-parallel over graphs; per core:
  A: layer-1 GINE on own nodes — host-staged [ea|1|x_src] columns, edge
     linear matmul per 128-edge block, relu, indicator-matmul scatter into
     PSUM (edges grouped per variable-boundary dst tile: <=128 nodes and
     <=256 edges, so every tile has exactly 2 blocks), node MLP.
  B: layer-1 recompute of h1[src] for every layer-2 edge slot ("virtual
     nodes" in C's stream order), grouped per C tile; output R node-major.
  C: layer-2 GINE — edge linear, R added via one identity matmul per tile,
     relu, scatter reusing A's indicators, node MLP, mean-pool partials;
     pool-reduce + MLP head at the end.
All matmul inputs bf16 (fp32 PSUM accumulate); head fp32. Streaming DMA
batched per super-tile and issued from the Pool-engine queue.
"""
import sys

sys.path.insert(0, "/opt/trn_rl_repo")

import math
from contextlib import ExitStack

import numpy as np

from concourse import bacc, bass, mybir, tile
from concourse import bass_utils
from concourse.masks import make_identity

F32 = mybir.dt.float32
BF16 = mybir.dt.bfloat16
I16 = mybir.dt.int16
AF = mybir.ActivationFunctionType
ALU = mybir.AluOpType
F32R = mybir.dt.float32r
NPBF = mybir.dt.np(BF16)
FR = lambda ap: ap.bitcast(F32R)

C = 8
P = 128
ND = 64
ED = 16
EMB = 128
USR = 12
KEXT = ED + 1 + ND     # [ea | 1 | x_src]
KE2 = ED + 1
G = 4096
NCH = 512
ETH = 256              # max edges per tile


def _wrap16(idx):
    L = len(idx)
    assert L % 16 == 0
    a = np.asarray(idx, np.int16).reshape(L // 16, 16).T
    return np.tile(a, (8, 1))


class CFG:
    pass


def _preprocess(x, edge_index, edge_attr, batch):
    x = np.asarray(x, np.float32)
    src = np.asarray(edge_index[0], np.int64)
    dst = np.asarray(edge_index[1], np.int64)
    batch = np.asarray(batch, np.int64)
    ea = np.asarray(edge_attr, np.float32)
    N = x.shape[0]

    GS = G // C
    gb = np.searchsorted(batch, np.arange(0, G + 1, GS))
    owner = np.searchsorted(gb, dst, side="right") - 1

    order_by_dst = np.argsort(dst, kind="stable")
    indeg = np.bincount(dst, minlength=N)
    in_start = np.concatenate([[0], np.cumsum(indeg)])

    # greedy variable tile boundaries per core: <=128 nodes, <=ETH edges
    cores = []
    for c in range(C):
        n_c = int(gb[c + 1] - gb[c])
        deg = indeg[gb[c]:gb[c + 1]]
        tile_of = np.zeros(n_c, np.int64)
        off_in = np.zeros(n_c, np.int64)
        t = nodes = edges = 0
        for v in range(n_c):
            if nodes >= P or edges + deg[v] > ETH:
                t += 1; nodes = 0; edges = 0
            tile_of[v] = t
            off_in[v] = nodes
            nodes += 1; edges += int(deg[v])
        cores.append(dict(n_c=n_c, tile_of=tile_of, off_in=off_in,
                          ntile=t + 1))

    NT = int(math.ceil(max(pc["ntile"] for pc in cores) / 4) * 4)
    N_SH = NT * P
    NST = NT // 4
    KA = ETH // P                      # blocks per tile (exactly 2)
    NBLK = NT * KA
    ESH_A = NBLK * P
    bo = np.arange(NT + 1) * KA

    # per-core edge -> slot
    for c, pc in enumerate(cores):
        em = np.nonzero(owner == c)[0]
        dloc = dst[em] - gb[c]
        t_of = pc["tile_of"][dloc]
        o = np.argsort(t_of, kind="stable")
        em, dloc, t_of = em[o], dloc[o], t_of[o]
        cnt_t = np.bincount(t_of, minlength=NT)
        assert cnt_t.max() <= ETH
        st_off = np.concatenate([[0], np.cumsum(cnt_t)])
        rank = np.arange(len(em)) - st_off[t_of]
        slot = t_of * ETH + rank
        pc.update(em=em, dloc=dloc, slot=slot)
        vsrc = np.full(ESH_A, -1, np.int64)
        vsrc[slot] = src[em]
        pc["vsrc"] = vsrc
        # node positions
        pos = pc["tile_of"] * P + pc["off_in"]
        pc["pos"] = pos

    # B group (= A/C block) edge counts
    kB = np.ones(NBLK, np.int64)
    for pc in cores:
        vs = pc["vsrc"]
        deg = np.where(vs >= 0, indeg[np.maximum(vs, 0)], 0)
        gcnt = deg.reshape(NBLK, P).sum(1)
        kB = np.maximum(kB, (gcnt + P - 1) // P)
    boB = np.concatenate([[0], np.cumsum(kB)])
    NBLKB = int(boB[-1])
    ESH_B = NBLKB * P

    cfg = CFG()
    cfg.N_SH, cfg.NT, cfg.NST, cfg.KA = N_SH, NT, NST, KA
    cfg.bo, cfg.NBLK, cfg.ESH_A = bo, NBLK, ESH_A
    cfg.kB, cfg.boB, cfg.NBLKB, cfg.ESH_B = kB, boB, NBLKB, ESH_B
    cfg.GS = GS
    cfg.GSP = max(P, int(math.ceil(GS / P) * P))
    assert kB.max() <= 8, kB.max()

    per_core = []
    for c, pc in enumerate(cores):
        em, slot, vsrc = pc["em"], pc["slot"], pc["vsrc"]
        n_c, pos = pc["n_c"], pc["pos"]
        eaExtA = np.zeros((KEXT, ESH_A), np.float32)
        eaExtA[:ED, slot] = ea[em].T
        eaExtA[ED, slot] = 1.0
        eaExtA[ED + 1:, slot] = x[src[em]].T
        dstrelA = np.full(ESH_A, 255.0, np.float32)
        dstrelA[slot] = pc["off_in"][pc["dloc"]].astype(np.float32)
        eaT2C = np.zeros((KE2, ESH_A), np.float32)
        eaT2C[:ED, slot] = ea[em].T
        eaT2C[ED, slot] = 1.0
        eaExtB = np.zeros((KEXT, ESH_B), np.float32)
        vrelB = np.full(ESH_B, 255.0, np.float32)
        xvT = np.zeros((ND, ESH_A), np.float32)
        real = vsrc >= 0
        xvT[:, real] = x[vsrc[real]].T
        deg = np.where(real, indeg[np.maximum(vsrc, 0)], 0)
        for g in range(NBLK):
            vv = vsrc[g * P:(g + 1) * P]
            dd = deg[g * P:(g + 1) * P]
            tot = int(dd.sum())
            if tot == 0:
                continue
            starts = in_start[np.maximum(vv, 0)]
            reps = np.repeat(starts, dd) + (
                np.arange(tot) - np.repeat(np.concatenate([[0], np.cumsum(dd)])[:-1], dd))
            eids = order_by_dst[reps]
            ppos = boB[g] * P + np.arange(tot)
            eaExtB[:ED, ppos] = ea[eids].T
            eaExtB[ED, ppos] = 1.0
            eaExtB[ED + 1:, ppos] = x[src[eids]].T
            vrelB[ppos] = np.repeat(np.arange(P), dd).astype(np.float32)

        # pooling structures on positions
        bl = batch[gb[c]:gb[c + 1]] - c * GS
        blp = np.full(N_SH, -1, np.int64)
        blp[pos] = bl
        tiles = blp.reshape(NT, P)
        g_first = np.array([t[t >= 0].min() if (t >= 0).any() else 0
                            for t in tiles])
        relg = np.where(blp >= 0, blp - np.repeat(g_first, P), 255.0)
        cnt = np.bincount(bl, minlength=GS).astype(np.float32)
        gstart = np.searchsorted(bl, np.arange(GS))
        gend = np.searchsorted(bl, np.arange(GS), side="right")
        t_lo = pc["tile_of"][np.minimum(gstart, n_c - 1)]
        t_hi = pc["tile_of"][np.maximum(gend - 1, gstart)]

        xT = np.zeros((ND, N_SH), np.float32)
        xT[:, pos] = x[gb[c]:gb[c + 1]].T

        per_core.append(dict(
            eaExtA=eaExtA, dstrelA=dstrelA.reshape(NBLK, P).T,
            eaT2C=eaT2C, eaExtB=eaExtB, vrelB=vrelB.reshape(NBLKB, P).T,
            xvT=xvT, n_c=n_c, xT=xT, pos=pos,
            relg=relg.astype(np.float32), g_first=g_first, cnt=cnt,
            t_lo=t_lo, t_hi=t_hi,
        ))

    cfg.GW = int(max((pc["relg"][pc["relg"] != 255.0]).max() + 1
                     if (pc["relg"] != 255.0).any() else 1 for pc in per_core))
    cfg.PG = int(max((pc["t_hi"] - pc["t_lo"] + 1)[pc["cnt"] > 0].max()
                     if (pc["cnt"] > 0).any() else 1 for pc in per_core))
    cfg.n_pool_idx = int(math.ceil(cfg.PG * cfg.GSP / P) * P)

    ZPAD = NT * cfg.GW
    for pc in per_core:
        pidx = np.full(cfg.n_pool_idx, ZPAD, np.int16)
        for g in range(GS):
            if pc["cnt"][g] <= 0:
                continue
            for p_, t in enumerate(range(pc["t_lo"][g], pc["t_hi"][g] + 1)):
                rel = g - pc["g_first"][t]
                pidx[p_ * cfg.GSP + g] = t * cfg.GW + rel
        pc["pool_idx"] = pidx
        pc["cnt_gm"] = np.maximum(
            np.pad(pc["cnt"], (0, cfg.GSP - GS)), 1.0
        ).reshape(cfg.GSP // P, P).T.astype(np.float32)

    relids = np.tile(np.arange(cfg.GW, dtype=np.float32), (P, 1))
    return cfg, gb, per_core, relids


def _build(cfg):
    nc = bacc.Bacc("TRN2", target_bir_lowering=False, debug=False,
                   num_devices=C)
    d = {}

    def inp(name, shape, dt=F32):
        d[name] = nc.dram_tensor(name, shape, dt, kind="ExternalInput").ap()

    NBLK, NBLKB, NT, NST, KA = cfg.NBLK, cfg.NBLKB, cfg.NT, cfg.NST, cfg.KA
    bo, kB, boB = cfg.bo, cfg.kB, cfg.boB
    GW, PG, GSP = cfg.GW, cfg.PG, cfg.GSP
    NROW = NT * GW + P

    inp("eaExtA", [KEXT, cfg.ESH_A], BF16)
    inp("eaExtB", [KEXT, cfg.ESH_B], BF16)
    inp("eaT2C", [KE2, cfg.ESH_A], BF16)
    inp("dstrelA", [P, NBLK], BF16)
    inp("vrelB", [P, NBLKB], BF16)
    inp("xvT", [ND, cfg.ESH_A])
    inp("xT", [ND, cfg.N_SH])
    inp("iota", [P, P], BF16)
    inp("W1e", [KEXT, ND], BF16)
    inp("W2eR", [ND + KE2, ND], BF16)
    inp("w11", [ND, ND], F32R)
    inp("b11", [ND, 1])
    inp("w12", [ND, ND], F32R)
    inp("b12", [ND, 1])
    inp("b12bf", [1, ND], BF16)
    inp("ones256", [1, ETH], BF16)
    inp("w21", [ND, EMB], F32R)
    inp("b21", [EMB, 1])
    inp("w22", [EMB, EMB], F32R)
    inp("b22", [EMB, 1])
    inp("relg", [P, NT], BF16)
    inp("relids", [P, GW], BF16)
    inp("pool_idx", [P, cfg.n_pool_idx // 16], I16)
    inp("cnt_gm", [P, GSP // P])
    inp("usrT", [USR, GSP])
    for nm, shp in (("hw1a", [EMB, 128]), ("hw1b", [USR, 128]), ("hb1", [128, 1]),
                    ("hw2", [128, 64]), ("hb2", [64, 1]), ("hw3", [64, 32]),
                    ("hb3", [32, 1]), ("hw4", [32, 16]), ("hb4", [16, 1]),
                    ("hw5", [16, 1]), ("hb5", [1, 1])):
        inp(nm, shp)
    yT = nc.dram_tensor("yT", [1, GSP], F32, kind="ExternalOutput").ap()

    with tile.TileContext(nc) as tc, ExitStack() as ctx:
        const = ctx.enter_context(tc.tile_pool(name="const", bufs=1))

        def ld(name, shape, dt=F32):
            t = const.tile(shape, dt, name=f"c_{name}")
            nc.sync.dma_start(t[:], d[name])
            return t

        W1e = ld("W1e", [KEXT, ND], BF16)
        W2eR = ld("W2eR", [ND + KE2, ND], BF16)
        w11 = ld("w11", [ND, ND], F32R); b11 = ld("b11", [ND, 1])
        w12 = ld("w12", [ND, ND], F32R); b12 = ld("b12", [ND, 1])
        b12bf = ld("b12bf", [1, ND], BF16)
        ones256 = ld("ones256", [1, ETH], BF16)
        w21 = ld("w21", [ND, EMB], F32R); b21 = ld("b21", [EMB, 1])
        w22 = ld("w22", [EMB, EMB], F32R); b22 = ld("b22", [EMB, 1])
        iota = ld("iota", [P, P], BF16)
        dstrelA = ld("dstrelA", [P, NBLK], BF16)
        vrelB = ld("vrelB", [P, NBLKB], BF16)
        xT = ld("xT", [ND, cfg.N_SH])
        relg = ld("relg", [P, NT], BF16)
        relids = ld("relids", [P, GW], BF16)
        ident = const.tile([P, P], F32, name="ident")
        make_identity(nc, ident[:])
        zt = const.tile([P, P], F32, name="zt")
        nc.vector.memset(zt[:], 0.0)

        dram = ctx.enter_context(tc.tile_pool(name="dram", bufs=1, space="DRAM"))
        parts = dram.tile([NROW, P], F32)
        nc.sync.dma_start(
            out=parts[NT * GW:NT * GW + P, :].rearrange("(p r) e -> p (r e)", p=P),
            in_=zt[:, :P])

        mctx = ctx.enter_context(ExitStack())
        psE = mctx.enter_context(tc.tile_pool(name="psE", bufs=3, space="PSUM"))
        psA = mctx.enter_context(tc.tile_pool(name="psA", bufs=3, space="PSUM"))
        psZ = mctx.enter_context(tc.tile_pool(name="psZ", bufs=2, space="PSUM"))
        eaAp = mctx.enter_context(tc.tile_pool(name="eaAp", bufs=2))
        eaBp = mctx.enter_context(tc.tile_pool(name="eaBp", bufs=2))
        ea2p = mctx.enter_context(tc.tile_pool(name="ea2p", bufs=2))
        xvp = mctx.enter_context(tc.tile_pool(name="xvp", bufs=2))
        msgp = mctx.enter_context(tc.tile_pool(name="msgp", bufs=3))
        indp = mctx.enter_context(tc.tile_pool(name="indp", bufs=12))
        hvp = mctx.enter_context(tc.tile_pool(name="hvp", bufs=3))
        z1p = mctx.enter_context(tc.tile_pool(name="z1p", bufs=3))
        hp = mctx.enter_context(tc.tile_pool(name="hp", bufs=2))
        h1p = mctx.enter_context(tc.tile_pool(name="h1p", bufs=2))
        embp = mctx.enter_context(tc.tile_pool(name="embp", bufs=2))
        s4p = mctx.enter_context(tc.tile_pool(name="s4p", bufs=2))
        psbp = mctx.enter_context(tc.tile_pool(name="psbp", bufs=2))

        for st in range(NST):
            t0, t1 = 4 * st, 4 * st + 4
            sA0, sA1 = bo[t0] * P, bo[t1] * P         # A/C slot range
            sB0, sB1 = boB[bo[t0]] * P, boB[bo[t1]] * P
            # ---- batched streaming loads for this super-tile ----
            eaA = eaAp.tile([KEXT, sA1 - sA0], BF16, tag="eaA")
            nc.gpsimd.dma_start(eaA[:], d["eaExtA"][:, sA0:sA1])
            eaB = eaBp.tile([KEXT, sB1 - sB0], BF16, tag="eaB")
            nc.gpsimd.dma_start(eaB[:], d["eaExtB"][:, sB0:sB1])
            e2R = ea2p.tile([ND + KE2, sA1 - sA0], BF16, tag="ea2")
            nc.gpsimd.dma_start(e2R[ND:, :], d["eaT2C"][:, sA0:sA1])
            xvt = xvp.tile([ND, sA1 - sA0], F32, tag="xvt")
            nc.gpsimd.dma_start(xvt[:], d["xvT"][:, sA0:sA1])

            ind_of = {}
            # ---- B: recompute R rows, one chunk per tile ----
            for t in range(t0, t1):
                L = KA * P
                co = (bo[t] - bo[t0]) * P              # chunk offset in xvt
                aggB = psA.tile([ND, L], F32, tag="agg")
                for gj in range(KA):
                    g = bo[t] + gj
                    kb = int(kB[g])
                    eL = psE.tile([P, kb * ND], F32, tag="eL")
                    msgB = msgp.tile([P, kb * ND], BF16, tag="msgB")
                    eoff = (boB[g] - boB[bo[t0]]) * P
                    for b in range(kb):
                        nc.tensor.matmul(
                            out=eL[:, b * ND:(b + 1) * ND],
                            lhsT=eaB[:, eoff + b * P:eoff + (b + 1) * P],
                            rhs=W1e[:], start=True, stop=True)
                    nc.scalar.activation(out=msgB[:], in_=eL[:], func=AF.Relu)
                    indB = indp.tile([P, kb * P], BF16, tag="indB")
                    nc.vector.tensor_tensor(
                        out=indB[:].rearrange("p (k e) -> p k e", e=P),
                        in0=vrelB[:, boB[g]:boB[g] + kb]
                        .unsqueeze(2).broadcast_to([P, kb, P]),
                        in1=iota[:].unsqueeze(1).broadcast_to([P, kb, P]),
                        op=ALU.is_equal)
                    for b in range(kb):
                        nc.tensor.matmul(out=aggB[:, gj * P:(gj + 1) * P],
                                         lhsT=msgB[:, b * ND:(b + 1) * ND],
                                         rhs=indB[:, b * P:(b + 1) * P],
                                         start=(b == 0), stop=(b == kb - 1))
                hv = hvp.tile([ND, L], F32R, tag="hv")
                nc.vector.tensor_add(out=hv[:], in0=aggB[:],
                                     in1=xvt[:, co:co + L])
                z1ps = psZ.tile([ND, L], F32, tag="z")
                nc.tensor.matmul(out=z1ps[:], lhsT=w11[:], rhs=hv[:],
                                 start=True, stop=True)
                z1sb = z1p.tile([ND, L], F32R, tag="z1B")
                nc.scalar.activation(out=z1sb[:], in_=z1ps[:], func=AF.Relu,
                                     bias=b11[:])
                Rps = psZ.tile([ND, L], F32, tag="z")
                nc.tensor.matmul(out=Rps[:], lhsT=w12[:], rhs=z1sb[:],
                                 start=True, stop=False)
                nc.tensor.matmul(out=Rps[:], lhsT=b12bf[:], rhs=ones256[:, :L],
                                 start=False, stop=True)
                nc.scalar.activation(out=e2R[:ND, co:co + L], in_=Rps[:],
                                     func=AF.Relu)
            # ---- A ----
            agg1 = psA.tile([ND, NCH], F32, tag="agg")
            for j, t in enumerate(range(t0, t1)):
                co = (bo[t] - bo[t0]) * P
                eL = psE.tile([P, KA * ND], F32, tag="eL")
                msgA = msgp.tile([P, KA * ND], BF16, tag="msgA")
                for b in range(KA):
                    nc.tensor.matmul(
                        out=eL[:, b * ND:(b + 1) * ND],
                        lhsT=eaA[:, co + b * P:co + (b + 1) * P],
                        rhs=W1e[:], start=True, stop=True)
                nc.scalar.activation(out=msgA[:], in_=eL[:], func=AF.Relu)
                indA = indp.tile([P, KA * P], BF16, tag="indA")
                nc.vector.tensor_tensor(
                    out=indA[:].rearrange("p (k e) -> p k e", e=P),
                    in0=dstrelA[:, bo[t]:bo[t] + KA]
                    .unsqueeze(2).broadcast_to([P, KA, P]),
                    in1=iota[:].unsqueeze(1).broadcast_to([P, KA, P]),
                    op=ALU.is_equal)
                ind_of[t] = indA
                for b in range(KA):
                    nc.tensor.matmul(out=agg1[:, j * P:(j + 1) * P],
                                     lhsT=msgA[:, b * ND:(b + 1) * ND],
                                     rhs=indA[:, b * P:(b + 1) * P],
                                     start=(b == 0), stop=(b == KA - 1))
            hT = hp.tile([ND, NCH], F32R, tag="hA")
            nc.vector.tensor_add(out=hT[:], in0=agg1[:],
                                 in1=xT[:, st * NCH:(st + 1) * NCH])
            z1ps = psZ.tile([ND, NCH], F32, tag="z")
            nc.tensor.matmul(out=z1ps[:], lhsT=w11[:], rhs=hT[:],
                             start=True, stop=True)
            z1sb = z1p.tile([ND, NCH], F32R, tag="z1A")
            nc.scalar.activation(out=z1sb[:], in_=z1ps[:], func=AF.Relu,
                                 bias=b11[:])
            h1ps = psZ.tile([ND, NCH], F32, tag="z")
            nc.tensor.matmul(out=h1ps[:], lhsT=w12[:], rhs=z1sb[:],
                             start=True, stop=True)
            h1T = h1p.tile([ND, NCH], F32, tag="h1T")
            nc.scalar.activation(out=h1T[:], in_=h1ps[:], func=AF.Relu,
                                 bias=b12[:])
            # ---- C ----
            agg2 = psA.tile([ND, NCH], F32, tag="agg")
            for j, t in enumerate(range(t0, t1)):
                co = (bo[t] - bo[t0]) * P
                eL2 = psE.tile([P, KA * ND], F32, tag="eL")
                for b in range(KA):
                    nc.tensor.matmul(
                        out=eL2[:, b * ND:(b + 1) * ND],
                        lhsT=e2R[:, co + b * P:co + (b + 1) * P],
                        rhs=W2eR[:], start=True, stop=True)
                msg2 = msgp.tile([P, KA * ND], BF16, tag="msg2")
                nc.scalar.activation(out=msg2[:], in_=eL2[:], func=AF.Relu)
                for b in range(KA):
                    nc.tensor.matmul(out=agg2[:, j * P:(j + 1) * P],
                                     lhsT=msg2[:, b * ND:(b + 1) * ND],
                                     rhs=ind_of[t][:, b * P:(b + 1) * P],
                                     start=(b == 0), stop=(b == KA - 1))
            hT2 = hp.tile([ND, NCH], F32R, tag="hC")
            nc.vector.tensor_add(out=hT2[:], in0=agg2[:], in1=h1T[:])
            z1ps2 = psZ.tile([EMB, NCH], F32, tag="z")
            nc.tensor.matmul(out=z1ps2[:], lhsT=w21[:], rhs=hT2[:],
                             start=True, stop=True)
            z1Csb = z1p.tile([EMB, NCH], F32R, tag="z1C")
            nc.scalar.activation(out=z1Csb[:], in_=z1ps2[:], func=AF.Relu,
                                 bias=b21[:])
            z2ps = psZ.tile([P, 4 * EMB], F32, tag="z")
            for j in range(4):
                nc.tensor.matmul(out=z2ps[:, j * EMB:(j + 1) * EMB],
                                 lhsT=z1Csb[:, j * P:(j + 1) * P],
                                 rhs=w22[:], start=True, stop=True)
            emb_nm = embp.tile([P, 4 * EMB], BF16, tag="emb")
            nc.scalar.activation(out=emb_nm[:], in_=z2ps[:], func=AF.Identity)
            S4 = s4p.tile([P, 4 * GW], BF16, tag="S4")
            nc.vector.tensor_tensor(
                out=S4[:].rearrange("p (k g) -> p k g", g=GW),
                in0=relg[:, t0:t1].unsqueeze(2).broadcast_to([P, 4, GW]),
                in1=relids[:].unsqueeze(1).broadcast_to([P, 4, GW]),
                op=ALU.is_equal)
            psb = psbp.tile([GW, 4 * P], F32, tag="psb")
            for j in range(4):
                pps = psZ.tile([GW, P], F32, tag="z")
                nc.tensor.matmul(out=pps[:], lhsT=S4[:, j * GW:(j + 1) * GW],
                                 rhs=emb_nm[:, j * EMB:(j + 1) * EMB],
                                 start=True, stop=True)
                nc.vector.tensor_copy(out=psb[:, j * P:(j + 1) * P], in_=pps[:])
            nc.gpsimd.dma_start(
                out=parts[t0 * GW:t1 * GW, :].rearrange("(k g) e -> g k e", g=GW),
                in_=psb[:].rearrange("g (k e) -> g k e", e=P))

        # ---- pool reduce + head ----
        mctx.close()
        with ExitStack() as hctx:
            hpool = hctx.enter_context(tc.tile_pool(name="hd", bufs=1))
            hps = hctx.enter_context(tc.tile_pool(name="hd_ps", bufs=2, space="PSUM"))
            pix = hpool.tile([P, cfg.n_pool_idx // 16], I16)
            nc.sync.dma_start(pix[:], d["pool_idx"])
            NPB = cfg.n_pool_idx // P
            gpo = hpool.tile([P, NPB * P], F32)
            nc.gpsimd.dma_gather(
                out_ap=gpo[:].rearrange("p (k e) -> p k e", e=P),
                in_ap=parts[:], idxs_ap=pix[:],
                num_idxs=cfg.n_pool_idx, num_idxs_reg=cfg.n_pool_idx,
                elem_size=P, single_packet=False)
            GB = GSP // P
            v = gpo[:].rearrange("p (q b e) -> p q b e", q=PG, b=GB)
            pooled = hpool.tile([P, GB * P], F32)
            pv = pooled[:].rearrange("p (b e) -> p b e", b=GB)
            if PG == 1:
                nc.vector.tensor_copy(out=pv, in_=v[:, 0])
            else:
                nc.vector.tensor_add(out=pv, in0=v[:, 0], in1=v[:, 1])
                for q in range(2, PG):
                    nc.vector.tensor_add(out=pv, in0=pv, in1=v[:, q])
            cntg = hpool.tile([P, GB], F32)
            nc.sync.dma_start(cntg[:], d["cnt_gm"])
            invc = hpool.tile([P, GB], F32)
            nc.vector.reciprocal(invc[:], cntg[:])
            for b in range(GB):
                nc.vector.tensor_tensor(
                    out=pooled[:, b * P:(b + 1) * P],
                    in0=pooled[:, b * P:(b + 1) * P],
                    in1=invc[:, b:b + 1].to_broadcast([P, P]), op=ALU.mult)
            embT = hpool.tile([P, GSP], F32)
            for b in range(GB):
                tps = hps.tile([P, P], F32, tag="hd")
                nc.tensor.transpose(out=tps[:], in_=pooled[:, b * P:(b + 1) * P],
                                    identity=ident[:])
                nc.vector.tensor_copy(out=embT[:, b * P:(b + 1) * P], in_=tps[:])
            nc.scalar.activation(out=embT[:], in_=embT[:], func=AF.Identity,
                                 bias=b22[:])
            usrT = hpool.tile([USR, GSP], F32)
            nc.sync.dma_start(usrT[:], d["usrT"])
            hw = {nm: hpool.tile(d[nm].shape, F32, name=f"t_{nm}")
                  for nm in ("hw1a", "hw1b", "hb1", "hw2", "hb2", "hw3", "hb3",
                             "hw4", "hb4", "hw5", "hb5")}
            for nm, t in hw.items():
                nc.sync.dma_start(t[:], d[nm])
            z1h = hps.tile([128, GSP], F32, tag="hd")
            nc.tensor.matmul(out=z1h[:], lhsT=hw["hw1a"][:], rhs=embT[:],
                             start=True, stop=False)
            nc.tensor.matmul(out=z1h[:], lhsT=hw["hw1b"][:], rhs=usrT[:],
                             start=False, stop=True)
            z1s = hpool.tile([128, GSP], F32)
            nc.scalar.activation(out=z1s[:], in_=z1h[:], func=AF.Relu, bias=hw["hb1"][:])
            z2h = hps.tile([64, GSP], F32, tag="hd")
            nc.tensor.matmul(out=z2h[:], lhsT=hw["hw2"][:], rhs=z1s[:], start=True, stop=True)
            z2s = hpool.tile([64, GSP], F32)
            nc.scalar.activation(out=z2s[:], in_=z2h[:], func=AF.Relu, bias=hw["hb2"][:])
            z3h = hps.tile([32, GSP], F32, tag="hd")
            nc.tensor.matmul(out=z3h[:], lhsT=hw["hw3"][:], rhs=z2s[:], start=True, stop=True)
            z3s = hpool.tile([32, GSP], F32)
            nc.scalar.activation(out=z3s[:], in_=z3h[:], func=AF.Relu, bias=hw["hb3"][:])
            z4h = hps.tile([16, GSP], F32, tag="hd")
            nc.tensor.matmul(out=z4h[:], lhsT=hw["hw4"][:], rhs=z3s[:], start=True, stop=True)
            z4s = hpool.tile([16, GSP], F32)
            nc.scalar.activation(out=z4s[:], in_=z4h[:], func=AF.Relu, bias=hw["hb4"][:])
            z5h = hps.tile([1, GSP], F32, tag="hd")
            nc.tensor.matmul(out=z5h[:], lhsT=hw["hw5"][:], rhs=z4s[:], start=True, stop=True)
            z5s = hpool.tile([1, GSP], F32)
            nc.scalar.activation(out=z5s[:], in_=z5h[:], func=AF.Identity, bias=hw["hb5"][:])
            nc.sync.dma_start(out=yT, in_=z5s[:])

    nc.compile()
    return nc


def _make_in_maps(cfg, gb, per_core, relids, inputs):
    f32 = lambda a: np.ascontiguousarray(np.asarray(a, np.float32))
    bf = lambda a: np.ascontiguousarray(np.asarray(a, np.float32)).astype(NPBF)
    W1ext = np.vstack([f32(inputs["e1_w"]), f32(inputs["e1_b"])[None, :],
                       np.eye(ND, dtype=np.float32)])
    W2ext = np.vstack([np.eye(ND, dtype=np.float32),
                       f32(inputs["e2_w"]), f32(inputs["e2_b"])[None, :]])
    usr = f32(inputs["usr"])
    iota = np.tile(np.arange(P, dtype=np.float32), (P, 1))
    in_maps = []
    for c, pc in enumerate(per_core):
        usrT = np.zeros((USR, cfg.GSP), np.float32)
        usrT[:, :cfg.GS] = usr[c * cfg.GS:(c + 1) * cfg.GS].T
        in_maps.append(dict(
            eaExtA=bf(pc["eaExtA"]), eaExtB=bf(pc["eaExtB"]),
            eaT2C=bf(pc["eaT2C"]), dstrelA=bf(pc["dstrelA"]),
            vrelB=bf(pc["vrelB"]), xvT=f32(pc["xvT"]),
            xT=f32(pc["xT"]), iota=bf(iota),
            W1e=bf(W1ext), W2eR=bf(W2ext),
            w11=f32(inputs["n1_w1"]), b11=f32(inputs["n1_b1"])[:, None],
            w12=f32(inputs["n1_w2"]), b12=f32(inputs["n1_b2"])[:, None],
            b12bf=bf(inputs["n1_b2"])[None, :],
            ones256=bf(np.ones((1, ETH))),
            w21=f32(inputs["n2_w1"]), b21=f32(inputs["n2_b1"])[:, None],
            w22=f32(inputs["n2_w2"]), b22=f32(inputs["n2_b2"])[:, None],
            relg=bf(pc["relg"].reshape(cfg.NT, P).T), relids=bf(relids),
            pool_idx=_wrap16(pc["pool_idx"]), cnt_gm=pc["cnt_gm"], usrT=usrT,
            hw1a=f32(inputs["h1_w"])[:EMB], hw1b=f32(inputs["h1_w"])[EMB:],
            hb1=f32(inputs["h1_b"])[:, None],
            hw2=f32(inputs["h2_w"]), hb2=f32(inputs["h2_b"])[:, None],
            hw3=f32(inputs["h3_w"]), hb3=f32(inputs["h3_b"])[:, None],
            hw4=f32(inputs["h4_w"]), hb4=f32(inputs["h4_b"])[:, None],
            hw5=f32(inputs["h5_w"]), hb5=f32(inputs["h5_b"])[:, None]))
    return in_maps


def kernel(**inputs):
    cfg, gb, per_core, relids = _preprocess(
        np.asarray(inputs["x"], np.float32), inputs["edge_index"],
        np.asarray(inputs["edge_attr"], np.float32), inputs["batch"])
    nc = _build(cfg)
    in_maps = _make_in_maps(cfg, gb, per_core, relids, inputs)
    res = bass_utils.run_bass_kernel_spmd(nc, in_maps, core_ids=list(range(C)))
    out = np.concatenate([res.results[c]["yT"][0, :cfg.GS] for c in range(C)])
    kernel._last = res
    return out[:, None].astype(np.float32)


# revision 4
# speedup vs baseline: 75.2799x; 1.1432x over previous
"""Trainium2 Bass kernel for nn_DockingTimeModel — dense-staged redesign v2.

Zero dynamic DMA, zero collectives. Data-parallel over graphs; per core:
  A: layer-1 GINE on own nodes — host-staged [ea|1|x_src] columns, edge
     linear matmul per 128-edge block, relu, indicator-matmul scatter into
     PSUM (edges grouped per variable-boundary dst tile: <=128 nodes and
     <=256 edges, so every tile has exactly 2 blocks), node MLP.
  B: layer-1 recompute of h1[src] for every layer-2 edge slot ("virtual
     nodes" in C's stream order), grouped per C tile; output R node-major.
  C: layer-2 GINE — edge linear, R added via one identity matmul per tile,
     relu, scatter reusing A's indicators, node MLP, mean-pool partials;
     pool-reduce + MLP head at the end.
All matmul inputs bf16 (fp32 PSUM accumulate); head fp32. Streaming DMA
batched per super-tile and issued from the Pool-engine queue.
"""
import sys

sys.path.insert(0, "/opt/trn_rl_repo")

import math
from contextlib import ExitStack

import numpy as np

from concourse import bacc, bass, mybir, tile
from concourse import bass_utils
from concourse.masks import make_identity

F32 = mybir.dt.float32
BF16 = mybir.dt.bfloat16
I16 = mybir.dt.int16
AF = mybir.ActivationFunctionType
ALU = mybir.AluOpType
F32R = mybir.dt.float32r
NPBF = mybir.dt.np(BF16)
FR = lambda ap: ap.bitcast(F32R)

C = 8
P = 128
ND = 64
ED = 16
EMB = 128
USR = 12
KEXT = ED + 1 + ND     # [ea | 1 | x_src]
KE2 = ED + 1
G = 4096
NCH = 512
ETH = 256              # max edges per tile


def _wrap16(idx):
    L = len(idx)
    assert L % 16 == 0
    a = np.asarray(idx, np.int16).reshape(L // 16, 16).T
    return np.tile(a, (8, 1))


class CFG:
    pass


def _preprocess(x, edge_index, edge_attr, batch):
    x = np.asarray(x, np.float32)
    src = np.asarray(edge_index[0], np.int64)
    dst = np.asarray(edge_index[1], np.int64)
    batch = np.asarray(batch, np.int64)
    ea = np.asarray(edge_attr, np.float32)
    N = x.shape[0]

    GS = G // C
    gb = np.searchsorted(batch, np.arange(0, G + 1, GS))
    owner = np.searchsorted(gb, dst, side="right") - 1

    order_by_dst = np.argsort(dst, kind="stable")
    indeg = np.bincount(dst, minlength=N)
    in_start = np.concatenate([[0], np.cumsum(indeg)])

    # greedy variable tile boundaries per core: <=128 nodes, <=ETH edges
    cores = []
    for c in range(C):
        n_c = int(gb[c + 1] - gb[c])
        deg = indeg[gb[c]:gb[c + 1]]
        tile_of = np.zeros(n_c, np.int64)
        off_in = np.zeros(n_c, np.int64)
        t = nodes = edges = 0
        for v in range(n_c):
            if nodes >= P or edges + deg[v] > ETH:
                t += 1; nodes = 0; edges = 0
            tile_of[v] = t
            off_in[v] = nodes
            nodes += 1; edges += int(deg[v])
        cores.append(dict(n_c=n_c, tile_of=tile_of, off_in=off_in,
                          ntile=t + 1))

    NT = int(math.ceil(max(pc["ntile"] for pc in cores) / 4) * 4)
    N_SH = NT * P
    NST = NT // 4
    KA = ETH // P                      # blocks per tile (exactly 2)
    NBLK = NT * KA
    ESH_A = NBLK * P
    bo = np.arange(NT + 1) * KA

    # per-core edge -> slot
    for c, pc in enumerate(cores):
        em = np.nonzero(owner == c)[0]
        dloc = dst[em] - gb[c]
        t_of = pc["tile_of"][dloc]
        o = np.argsort(t_of, kind="stable")
        em, dloc, t_of = em[o], dloc[o], t_of[o]
        cnt_t = np.bincount(t_of, minlength=NT)
        assert cnt_t.max() <= ETH
        st_off = np.concatenate([[0], np.cumsum(cnt_t)])
        rank = np.arange(len(em)) - st_off[t_of]
        slot = t_of * ETH + rank
        pc.update(em=em, dloc=dloc, slot=slot)
        vsrc = np.full(ESH_A, -1, np.int64)
        vsrc[slot] = src[em]
        pc["vsrc"] = vsrc
        # node positions
        pos = pc["tile_of"] * P + pc["off_in"]
        pc["pos"] = pos

    # B group (= A/C block) edge counts
    kB = np.ones(NBLK, np.int64)
    for pc in cores:
        vs = pc["vsrc"]
        deg = np.where(vs >= 0, indeg[np.maximum(vs, 0)], 0)
        gcnt = deg.reshape(NBLK, P).sum(1)
        kB = np.maximum(kB, (gcnt + P - 1) // P)
    boB = np.concatenate([[0], np.cumsum(kB)])
    NBLKB = int(boB[-1])
    ESH_B = NBLKB * P

    cfg = CFG()
    cfg.N_SH, cfg.NT, cfg.NST, cfg.KA = N_SH, NT, NST, KA
    cfg.bo, cfg.NBLK, cfg.ESH_A = bo, NBLK, ESH_A
    cfg.kB, cfg.boB, cfg.NBLKB, cfg.ESH_B = kB, boB, NBLKB, ESH_B
    cfg.GS = GS
    cfg.GSP = max(P, int(math.ceil(GS / P) * P))
    assert kB.max() <= 8, kB.max()

    per_core = []
    for c, pc in enumerate(cores):
        em, slot, vsrc = pc["em"], pc["slot"], pc["vsrc"]
        n_c, pos = pc["n_c"], pc["pos"]
        eaExtA = np.zeros((KEXT, ESH_A), np.float32)
        eaExtA[:ED, slot] = ea[em].T
        eaExtA[ED, slot] = 1.0
        eaExtA[ED + 1:, slot] = x[src[em]].T
        dstrelA = np.full(ESH_A, 255.0, np.float32)
        dstrelA[slot] = pc["off_in"][pc["dloc"]].astype(np.float32)
        eaT2C = np.zeros((KE2, ESH_A), np.float32)
        eaT2C[:ED, slot] = ea[em].T
        eaT2C[ED, slot] = 1.0
        eaExtB = np.zeros((KEXT, ESH_B), np.float32)
        vrelB = np.full(ESH_B, 255.0, np.float32)
        xvT = np.zeros((ND, ESH_A), np.float32)
        real = vsrc >= 0
        xvT[:, real] = x[vsrc[real]].T
        deg = np.where(real, indeg[np.maximum(vsrc, 0)], 0)
        for g in range(NBLK):
            vv = vsrc[g * P:(g + 1) * P]
            dd = deg[g * P:(g + 1) * P]
            tot = int(dd.sum())
            if tot == 0:
                continue
            starts = in_start[np.maximum(vv, 0)]
            reps = np.repeat(starts, dd) + (
                np.arange(tot) - np.repeat(np.concatenate([[0], np.cumsum(dd)])[:-1], dd))
            eids = order_by_dst[reps]
            ppos = boB[g] * P + np.arange(tot)
            eaExtB[:ED, ppos] = ea[eids].T
            eaExtB[ED, ppos] = 1.0
            eaExtB[ED + 1:, ppos] = x[src[eids]].T
            vrelB[ppos] = np.repeat(np.arange(P), dd).astype(np.float32)

        # pooling structures on positions
        bl = batch[gb[c]:gb[c + 1]] - c * GS
        blp = np.full(N_SH, -1, np.int64)
        blp[pos] = bl
        tiles = blp.reshape(NT, P)
        g_first = np.array([t[t >= 0].min() if (t >= 0).any() else 0
                            for t in tiles])
        relg = np.where(blp >= 0, blp - np.repeat(g_first, P), 255.0)
        cnt = np.bincount(bl, minlength=GS).astype(np.float32)
        gstart = np.searchsorted(bl, np.arange(GS))
        gend = np.searchsorted(bl, np.arange(GS), side="right")
        t_lo = pc["tile_of"][np.minimum(gstart, n_c - 1)]
        t_hi = pc["tile_of"][np.maximum(gend - 1, gstart)]

        xT = np.zeros((ND, N_SH), np.float32)
        xT[:, pos] = x[gb[c]:gb[c + 1]].T

        per_core.append(dict(
            eaExtA=eaExtA, dstrelA=dstrelA.reshape(NBLK, P).T,
            eaT2C=eaT2C, eaExtB=eaExtB, vrelB=vrelB.reshape(NBLKB, P).T,
            xvT=xvT, n_c=n_c, xT=xT, pos=pos,
            relg=relg.astype(np.float32), g_first=g_first, cnt=cnt,
            t_lo=t_lo, t_hi=t_hi,
        ))

    cfg.GW = int(max((pc["relg"][pc["relg"] != 255.0]).max() + 1
                     if (pc["relg"] != 255.0).any() else 1 for pc in per_core))
    cfg.PG = int(max((pc["t_hi"] - pc["t_lo"] + 1)[pc["cnt"] > 0].max()
                     if (pc["cnt"] > 0).any() else 1 for pc in per_core))
    cfg.n_pool_idx = int(math.ceil(cfg.PG * cfg.GSP / P) * P)

    ZPAD = NT * cfg.GW
    for pc in per_core:
        pidx = np.full(cfg.n_pool_idx, ZPAD, np.int16)
        for g in range(GS):
            if pc["cnt"][g] <= 0:
                continue
            for p_, t in enumerate(range(pc["t_lo"][g], pc["t_hi"][g] + 1)):
                rel = g - pc["g_first"][t]
                pidx[p_ * cfg.GSP + g] = t * cfg.GW + rel
        pc["pool_idx"] = pidx
        pc["cnt_gm"] = np.maximum(
            np.pad(pc["cnt"], (0, cfg.GSP - GS)), 1.0
        ).reshape(cfg.GSP // P, P).T.astype(np.float32)

    relids = np.tile(np.arange(cfg.GW, dtype=np.float32), (P, 1))
    return cfg, gb, per_core, relids


def _build(cfg):
    nc = bacc.Bacc("TRN2", target_bir_lowering=False, debug=False,
                   num_devices=C)
    d = {}

    def inp(name, shape, dt=F32):
        d[name] = nc.dram_tensor(name, shape, dt, kind="ExternalInput").ap()

    NBLK, NBLKB, NT, NST, KA = cfg.NBLK, cfg.NBLKB, cfg.NT, cfg.NST, cfg.KA
    bo, kB, boB = cfg.bo, cfg.kB, cfg.boB
    GW, PG, GSP = cfg.GW, cfg.PG, cfg.GSP
    NROW = NT * GW + P

    inp("eaExtA", [KEXT, cfg.ESH_A], BF16)
    inp("eaExtB", [KEXT, cfg.ESH_B], BF16)
    inp("eaT2C", [KE2, cfg.ESH_A], BF16)
    inp("dstrelA", [P, NBLK], BF16)
    inp("vrelB", [P, NBLKB], BF16)
    inp("xvT", [ND, cfg.ESH_A])
    inp("xT", [ND, cfg.N_SH])
    inp("iota", [P, P], BF16)
    inp("W1e", [KEXT, ND], BF16)
    inp("W2eR", [ND + KE2, ND], BF16)
    inp("w11", [ND, ND], F32R)
    inp("b11", [ND, 1])
    inp("w12", [ND, ND], F32R)
    inp("b12", [ND, 1])
    inp("w21", [ND, EMB], F32R)
    inp("b21", [EMB, 1])
    inp("w22", [EMB, EMB], F32R)
    inp("b22", [EMB, 1])
    inp("relg", [P, NT], BF16)
    inp("relids", [P, GW], BF16)
    inp("pool_idx", [P, cfg.n_pool_idx // 16], I16)
    inp("cnt_gm", [P, GSP // P])
    inp("usrT", [USR, GSP])
    for nm, shp in (("hw1a", [EMB, 128]), ("hw1b", [USR, 128]), ("hb1", [128, 1]),
                    ("hw2", [128, 64]), ("hb2", [64, 1]), ("hw3", [64, 32]),
                    ("hb3", [32, 1]), ("hw4", [32, 16]), ("hb4", [16, 1]),
                    ("hw5", [16, 1]), ("hb5", [1, 1])):
        inp(nm, shp)
    yT = nc.dram_tensor("yT", [1, GSP], F32, kind="ExternalOutput").ap()

    with tile.TileContext(nc) as tc, ExitStack() as ctx:
        const = ctx.enter_context(tc.tile_pool(name="const", bufs=1))

        def ld(name, shape, dt=F32):
            t = const.tile(shape, dt, name=f"c_{name}")
            nc.sync.dma_start(t[:], d[name])
            return t

        W1e = ld("W1e", [KEXT, ND], BF16)
        W2eR = ld("W2eR", [ND + KE2, ND], BF16)
        w11 = ld("w11", [ND, ND], F32R); b11 = ld("b11", [ND, 1])
        w12 = ld("w12", [ND, ND], F32R); b12 = ld("b12", [ND, 1])
        w21 = ld("w21", [ND, EMB], F32R); b21 = ld("b21", [EMB, 1])
        w22 = ld("w22", [EMB, EMB], F32R); b22 = ld("b22", [EMB, 1])
        iota = ld("iota", [P, P], BF16)
        dstrelA = ld("dstrelA", [P, NBLK], BF16)
        vrelB = ld("vrelB", [P, NBLKB], BF16)
        xT = ld("xT", [ND, cfg.N_SH])
        relg = ld("relg", [P, NT], BF16)
        relids = ld("relids", [P, GW], BF16)
        ident = const.tile([P, P], F32, name="ident")
        make_identity(nc, ident[:])
        zt = const.tile([P, P], F32, name="zt")
        nc.vector.memset(zt[:], 0.0)

        dram = ctx.enter_context(tc.tile_pool(name="dram", bufs=1, space="DRAM"))
        parts = dram.tile([NROW, P], F32)
        nc.sync.dma_start(
            out=parts[NT * GW:NT * GW + P, :].rearrange("(p r) e -> p (r e)", p=P),
            in_=zt[:, :P])

        mctx = ctx.enter_context(ExitStack())
        psE = mctx.enter_context(tc.tile_pool(name="psE", bufs=3, space="PSUM"))
        psA = mctx.enter_context(tc.tile_pool(name="psA", bufs=3, space="PSUM"))
        psZ = mctx.enter_context(tc.tile_pool(name="psZ", bufs=2, space="PSUM"))
        eaAp = mctx.enter_context(tc.tile_pool(name="eaAp", bufs=2))
        eaBp = mctx.enter_context(tc.tile_pool(name="eaBp", bufs=2))
        ea2p = mctx.enter_context(tc.tile_pool(name="ea2p", bufs=2))
        xvp = mctx.enter_context(tc.tile_pool(name="xvp", bufs=2))
        msgp = mctx.enter_context(tc.tile_pool(name="msgp", bufs=3))
        indp = mctx.enter_context(tc.tile_pool(name="indp", bufs=12))
        hvp = mctx.enter_context(tc.tile_pool(name="hvp", bufs=3))
        z1p = mctx.enter_context(tc.tile_pool(name="z1p", bufs=3))
        hp = mctx.enter_context(tc.tile_pool(name="hp", bufs=2))
        h1p = mctx.enter_context(tc.tile_pool(name="h1p", bufs=2))
        embp = mctx.enter_context(tc.tile_pool(name="embp", bufs=2))
        s4p = mctx.enter_context(tc.tile_pool(name="s4p", bufs=2))
        psbp = mctx.enter_context(tc.tile_pool(name="psbp", bufs=2))

        for st in range(NST):
            t0, t1 = 4 * st, 4 * st + 4
            sA0, sA1 = bo[t0] * P, bo[t1] * P         # A/C slot range
            sB0, sB1 = boB[bo[t0]] * P, boB[bo[t1]] * P
            # ---- batched streaming loads for this super-tile ----
            eaA = eaAp.tile([KEXT, sA1 - sA0], BF16, tag="eaA")
            nc.gpsimd.dma_start(eaA[:], d["eaExtA"][:, sA0:sA1])
            eaB = eaBp.tile([KEXT, sB1 - sB0], BF16, tag="eaB")
            nc.gpsimd.dma_start(eaB[:], d["eaExtB"][:, sB0:sB1])
            e2R = ea2p.tile([ND + KE2, sA1 - sA0], BF16, tag="ea2")
            nc.gpsimd.dma_start(e2R[ND:, :], d["eaT2C"][:, sA0:sA1])
            xvt = xvp.tile([ND, sA1 - sA0], F32, tag="xvt")
            nc.gpsimd.dma_start(xvt[:], d["xvT"][:, sA0:sA1])

            ind_of = {}
            # ---- B: recompute R rows, tile-pair chunks ----
            for pr in range(2):
                tp0 = t0 + 2 * pr
                L = KA * P
                L2 = 2 * L
                co2 = (bo[tp0] - bo[t0]) * P
                aggBs = []
                for t in (tp0, tp0 + 1):
                    aggB = psA.tile([ND, L], F32, tag="agg")
                    for gj in range(KA):
                        g = bo[t] + gj
                        kb = int(kB[g])
                        eL = psE.tile([P, kb * ND], F32, tag="eL")
                        msgB = msgp.tile([P, kb * ND], BF16, tag="msgB")
                        eoff = (boB[g] - boB[bo[t0]]) * P
                        for b in range(kb):
                            nc.tensor.matmul(
                                out=eL[:, b * ND:(b + 1) * ND],
                                lhsT=eaB[:, eoff + b * P:eoff + (b + 1) * P],
                                rhs=W1e[:], start=True, stop=True)
                        nc.scalar.activation(out=msgB[:], in_=eL[:], func=AF.Relu)
                        indB = indp.tile([P, kb * P], BF16, tag="indB")
                        nc.vector.tensor_tensor(
                            out=indB[:].rearrange("p (k e) -> p k e", e=P),
                            in0=vrelB[:, boB[g]:boB[g] + kb]
                            .unsqueeze(2).broadcast_to([P, kb, P]),
                            in1=iota[:].unsqueeze(1).broadcast_to([P, kb, P]),
                            op=ALU.is_equal)
                        for b in range(kb):
                            nc.tensor.matmul(out=aggB[:, gj * P:(gj + 1) * P],
                                             lhsT=msgB[:, b * ND:(b + 1) * ND],
                                             rhs=indB[:, b * P:(b + 1) * P],
                                             start=(b == 0), stop=(b == kb - 1))
                    aggBs.append(aggB)
                hv = hvp.tile([ND, L2], F32R, tag="hv")
                for i in range(2):
                    nc.vector.tensor_add(
                        out=hv[:, i * L:(i + 1) * L], in0=aggBs[i][:],
                        in1=xvt[:, co2 + i * L:co2 + (i + 1) * L])
                z1ps = psZ.tile([ND, L2], F32, tag="z")
                nc.tensor.matmul(out=z1ps[:], lhsT=w11[:], rhs=hv[:],
                                 start=True, stop=True)
                z1sb = z1p.tile([ND, L2], F32R, tag="z1B")
                nc.scalar.activation(out=z1sb[:], in_=z1ps[:], func=AF.Relu,
                                     bias=b11[:])
                Rps = psZ.tile([ND, L2], F32, tag="z")
                nc.tensor.matmul(out=Rps[:], lhsT=w12[:], rhs=z1sb[:],
                                 start=True, stop=True)
                nc.scalar.activation(out=e2R[:ND, co2:co2 + L2], in_=Rps[:],
                                     func=AF.Relu, bias=b12[:])
            # ---- A ----
            agg1 = psA.tile([ND, NCH], F32, tag="agg")
            for j, t in enumerate(range(t0, t1)):
                co = (bo[t] - bo[t0]) * P
                eL = psE.tile([P, KA * ND], F32, tag="eL")
                msgA = msgp.tile([P, KA * ND], BF16, tag="msgA")
                for b in range(KA):
                    nc.tensor.matmul(
                        out=eL[:, b * ND:(b + 1) * ND],
                        lhsT=eaA[:, co + b * P:co + (b + 1) * P],
                        rhs=W1e[:], start=True, stop=True)
                nc.scalar.activation(out=msgA[:], in_=eL[:], func=AF.Relu)
                indA = indp.tile([P, KA * P], BF16, tag="indA")
                nc.vector.tensor_tensor(
                    out=indA[:].rearrange("p (k e) -> p k e", e=P),
                    in0=dstrelA[:, bo[t]:bo[t] + KA]
                    .unsqueeze(2).broadcast_to([P, KA, P]),
                    in1=iota[:].unsqueeze(1).broadcast_to([P, KA, P]),
                    op=ALU.is_equal)
                ind_of[t] = indA
                for b in range(KA):
                    nc.tensor.matmul(out=agg1[:, j * P:(j + 1) * P],
                                     lhsT=msgA[:, b * ND:(b + 1) * ND],
                                     rhs=indA[:, b * P:(b + 1) * P],
                                     start=(b == 0), stop=(b == KA - 1))
            hT = hp.tile([ND, NCH], F32R, tag="hA")
            nc.vector.tensor_add(out=hT[:], in0=agg1[:],
                                 in1=xT[:, st * NCH:(st + 1) * NCH])
            z1ps = psZ.tile([ND, NCH], F32, tag="z")
            nc.tensor.matmul(out=z1ps[:], lhsT=w11[:], rhs=hT[:],
                             start=True, stop=True)
            z1sb = z1p.tile([ND, NCH], F32R, tag="z1A")
            nc.scalar.activation(out=z1sb[:], in_=z1ps[:], func=AF.Relu,
                                 bias=b11[:])
            h1ps = psZ.tile([ND, NCH], F32, tag="z")
            nc.tensor.matmul(out=h1ps[:], lhsT=w12[:], rhs=z1sb[:],
                             start=True, stop=True)
            h1T = h1p.tile([ND, NCH], F32, tag="h1T")
            nc.scalar.activation(out=h1T[:], in_=h1ps[:], func=AF.Relu,
                                 bias=b12[:])
            # ---- C ----
            agg2 = psA.tile([ND, NCH], F32, tag="agg")
            for j, t in enumerate(range(t0, t1)):
                co = (bo[t] - bo[t0]) * P
                eL2 = psE.tile([P, KA * ND], F32, tag="eL")
                for b in range(KA):
                    nc.tensor.matmul(
                        out=eL2[:, b * ND:(b + 1) * ND],
                        lhsT=e2R[:, co + b * P:co + (b + 1) * P],
                        rhs=W2eR[:], start=True, stop=True)
                msg2 = msgp.tile([P, KA * ND], BF16, tag="msg2")
                nc.scalar.activation(out=msg2[:], in_=eL2[:], func=AF.Relu)
                for b in range(KA):
                    nc.tensor.matmul(out=agg2[:, j * P:(j + 1) * P],
                                     lhsT=msg2[:, b * ND:(b + 1) * ND],
                                     rhs=ind_of[t][:, b * P:(b + 1) * P],
                                     start=(b == 0), stop=(b == KA - 1))
            hT2 = hp.tile([ND, NCH], F32R, tag="hC")
            nc.vector.tensor_add(out=hT2[:], in0=agg2[:], in1=h1T[:])
            z1ps2 = psZ.tile([EMB, NCH], F32, tag="z")
            nc.tensor.matmul(out=z1ps2[:], lhsT=w21[:], rhs=hT2[:],
                             start=True, stop=True)
            z1Csb = z1p.tile([EMB, NCH], F32R, tag="z1C")
            nc.scalar.activation(out=z1Csb[:], in_=z1ps2[:], func=AF.Relu,
                                 bias=b21[:])
            z2ps = psZ.tile([P, 4 * EMB], F32, tag="z")
            for j in range(4):
                nc.tensor.matmul(out=z2ps[:, j * EMB:(j + 1) * EMB],
                                 lhsT=z1Csb[:, j * P:(j + 1) * P],
                                 rhs=w22[:], start=True, stop=True)
            emb_nm = embp.tile([P, 4 * EMB], BF16, tag="emb")
            nc.scalar.activation(out=emb_nm[:], in_=z2ps[:], func=AF.Identity)
            S4 = s4p.tile([P, 4 * GW], BF16, tag="S4")
            nc.vector.tensor_tensor(
                out=S4[:].rearrange("p (k g) -> p k g", g=GW),
                in0=relg[:, t0:t1].unsqueeze(2).broadcast_to([P, 4, GW]),
                in1=relids[:].unsqueeze(1).broadcast_to([P, 4, GW]),
                op=ALU.is_equal)
            psb = psbp.tile([GW, 4 * P], F32, tag="psb")
            for j in range(4):
                pps = psZ.tile([GW, P], F32, tag="z")
                nc.tensor.matmul(out=pps[:], lhsT=S4[:, j * GW:(j + 1) * GW],
                                 rhs=emb_nm[:, j * EMB:(j + 1) * EMB],
                                 start=True, stop=True)
                nc.vector.tensor_copy(out=psb[:, j * P:(j + 1) * P], in_=pps[:])
            nc.gpsimd.dma_start(
                out=parts[t0 * GW:t1 * GW, :].rearrange("(k g) e -> g k e", g=GW),
                in_=psb[:].rearrange("g (k e) -> g k e", e=P))

        # ---- pool reduce + head ----
        mctx.close()
        with ExitStack() as hctx:
            hpool = hctx.enter_context(tc.tile_pool(name="hd", bufs=1))
            hps = hctx.enter_context(tc.tile_pool(name="hd_ps", bufs=2, space="PSUM"))
            pix = hpool.tile([P, cfg.n_pool_idx // 16], I16)
            nc.sync.dma_start(pix[:], d["pool_idx"])
            NPB = cfg.n_pool_idx // P
            gpo = hpool.tile([P, NPB * P], F32)
            nc.gpsimd.dma_gather(
                out_ap=gpo[:].rearrange("p (k e) -> p k e", e=P),
                in_ap=parts[:], idxs_ap=pix[:],
                num_idxs=cfg.n_pool_idx, num_idxs_reg=cfg.n_pool_idx,
                elem_size=P, single_packet=False)
            GB = GSP // P
            v = gpo[:].rearrange("p (q b e) -> p q b e", q=PG, b=GB)
            pooled = hpool.tile([P, GB * P], F32)
            pv = pooled[:].rearrange("p (b e) -> p b e", b=GB)
            if PG == 1:
                nc.vector.tensor_copy(out=pv, in_=v[:, 0])
            else:
                nc.vector.tensor_add(out=pv, in0=v[:, 0], in1=v[:, 1])
                for q in range(2, PG):
                    nc.vector.tensor_add(out=pv, in0=pv, in1=v[:, q])
            cntg = hpool.tile([P, GB], F32)
            nc.sync.dma_start(cntg[:], d["cnt_gm"])
            invc = hpool.tile([P, GB], F32)
            nc.vector.reciprocal(invc[:], cntg[:])
            for b in range(GB):
                nc.vector.tensor_tensor(
                    out=pooled[:, b * P:(b + 1) * P],
                    in0=pooled[:, b * P:(b + 1) * P],
                    in1=invc[:, b:b + 1].to_broadcast([P, P]), op=ALU.mult)
            embT = hpool.tile([P, GSP], F32)
            for b in range(GB):
                tps = hps.tile([P, P], F32, tag="hd")
                nc.tensor.transpose(out=tps[:], in_=pooled[:, b * P:(b + 1) * P],
                                    identity=ident[:])
                nc.vector.tensor_copy(out=embT[:, b * P:(b + 1) * P], in_=tps[:])
            nc.scalar.activation(out=embT[:], in_=embT[:], func=AF.Identity,
                                 bias=b22[:])
            usrT = hpool.tile([USR, GSP], F32)
            nc.sync.dma_start(usrT[:], d["usrT"])
            hw = {nm: hpool.tile(d[nm].shape, F32, name=f"t_{nm}")
                  for nm in ("hw1a", "hw1b", "hb1", "hw2", "hb2", "hw3", "hb3",
                             "hw4", "hb4", "hw5", "hb5")}
            for nm, t in hw.items():
                nc.sync.dma_start(t[:], d[nm])
            z1h = hps.tile([128, GSP], F32, tag="hd")
            nc.tensor.matmul(out=z1h[:], lhsT=hw["hw1a"][:], rhs=embT[:],
                             start=True, stop=False)
            nc.tensor.matmul(out=z1h[:], lhsT=hw["hw1b"][:], rhs=usrT[:],
                             start=False, stop=True)
            z1s = hpool.tile([128, GSP], F32)
            nc.scalar.activation(out=z1s[:], in_=z1h[:], func=AF.Relu, bias=hw["hb1"][:])
            z2h = hps.tile([64, GSP], F32, tag="hd")
            nc.tensor.matmul(out=z2h[:], lhsT=hw["hw2"][:], rhs=z1s[:], start=True, stop=True)
            z2s = hpool.tile([64, GSP], F32)
            nc.scalar.activation(out=z2s[:], in_=z2h[:], func=AF.Relu, bias=hw["hb2"][:])
            z3h = hps.tile([32, GSP], F32, tag="hd")
            nc.tensor.matmul(out=z3h[:], lhsT=hw["hw3"][:], rhs=z2s[:], start=True, stop=True)
            z3s = hpool.tile([32, GSP], F32)
            nc.scalar.activation(out=z3s[:], in_=z3h[:], func=AF.Relu, bias=hw["hb3"][:])
            z4h = hps.tile([16, GSP], F32, tag="hd")
            nc.tensor.matmul(out=z4h[:], lhsT=hw["hw4"][:], rhs=z3s[:], start=True, stop=True)
            z4s = hpool.tile([16, GSP], F32)
            nc.scalar.activation(out=z4s[:], in_=z4h[:], func=AF.Relu, bias=hw["hb4"][:])
            z5h = hps.tile([1, GSP], F32, tag="hd")
            nc.tensor.matmul(out=z5h[:], lhsT=hw["hw5"][:], rhs=z4s[:], start=True, stop=True)
            z5s = hpool.tile([1, GSP], F32)
            nc.scalar.activation(out=z5s[:], in_=z5h[:], func=AF.Identity, bias=hw["hb5"][:])
            nc.sync.dma_start(out=yT, in_=z5s[:])

    nc.compile()
    return nc


def _make_in_maps(cfg, gb, per_core, relids, inputs):
    f32 = lambda a: np.ascontiguousarray(np.asarray(a, np.float32))
    bf = lambda a: np.ascontiguousarray(np.asarray(a, np.float32)).astype(NPBF)
    W1ext = np.vstack([f32(inputs["e1_w"]), f32(inputs["e1_b"])[None, :],
                       np.eye(ND, dtype=np.float32)])
    W2ext = np.vstack([np.eye(ND, dtype=np.float32),
                       f32(inputs["e2_w"]), f32(inputs["e2_b"])[None, :]])
    usr = f32(inputs["usr"])
    iota = np.tile(np.arange(P, dtype=np.float32), (P, 1))
    in_maps = []
    for c, pc in enumerate(per_core):
        usrT = np.zeros((USR, cfg.GSP), np.float32)
        usrT[:, :cfg.GS] = usr[c * cfg.GS:(c + 1) * cfg.GS].T
        in_maps.append(dict(
            eaExtA=bf(pc["eaExtA"]), eaExtB=bf(pc["eaExtB"]),
            eaT2C=bf(pc["eaT2C"]), dstrelA=bf(pc["dstrelA"]),
            vrelB=bf(pc["vrelB"]), xvT=f32(pc["xvT"]),
            xT=f32(pc["xT"]), iota=bf(iota),
            W1e=bf(W1ext), W2eR=bf(W2ext),
            w11=f32(inputs["n1_w1"]), b11=f32(inputs["n1_b1"])[:, None],
            w12=f32(inputs["n1_w2"]), b12=f32(inputs["n1_b2"])[:, None],

            w21=f32(inputs["n2_w1"]), b21=f32(inputs["n2_b1"])[:, None],
            w22=f32(inputs["n2_w2"]), b22=f32(inputs["n2_b2"])[:, None],
            relg=bf(pc["relg"].reshape(cfg.NT, P).T), relids=bf(relids),
            pool_idx=_wrap16(pc["pool_idx"]), cnt_gm=pc["cnt_gm"], usrT=usrT,
            hw1a=f32(inputs["h1_w"])[:EMB], hw1b=f32(inputs["h1_w"])[EMB:],
            hb1=f32(inputs["h1_b"])[:, None],
            hw2=f32(inputs["h2_w"]), hb2=f32(inputs["h2_b"])[:, None],
            hw3=f32(inputs["h3_w"]), hb3=f32(inputs["h3_b"])[:, None],
            hw4=f32(inputs["h4_w"]), hb4=f32(inputs["h4_b"])[:, None],
            hw5=f32(inputs["h5_w"]), hb5=f32(inputs["h5_b"])[:, None]))
    return in_maps


def kernel(**inputs):
    cfg, gb, per_core, relids = _preprocess(
        np.asarray(inputs["x"], np.float32), inputs["edge_index"],
        np.asarray(inputs["edge_attr"], np.float32), inputs["batch"])
    nc = _build(cfg)
    in_maps = _make_in_maps(cfg, gb, per_core, relids, inputs)
    res = bass_utils.run_bass_kernel_spmd(nc, in_maps, core_ids=list(range(C)))
    out = np.concatenate([res.results[c]["yT"][0, :cfg.GS] for c in range(C)])
    kernel._last = res
    return out[:, None].astype(np.float32)


# revision 5
# speedup vs baseline: 77.1843x; 1.0253x over previous
"""Trainium2 Bass kernel for nn_DockingTimeModel — dense-staged redesign v2.

Zero dynamic DMA, zero collectives. Data-parallel over graphs; per core:
  A: layer-1 GINE on own nodes — host-staged [ea|1|x_src] columns, edge
     linear matmul per 128-edge block, relu, indicator-matmul scatter into
     PSUM (edges grouped per variable-boundary dst tile: <=128 nodes and
     <=256 edges, so every tile has exactly 2 blocks), node MLP.
  B: layer-1 recompute of h1[src] for every layer-2 edge slot ("virtual
     nodes" in C's stream order), grouped per C tile; output R node-major.
  C: layer-2 GINE — edge linear, R added via one identity matmul per tile,
     relu, scatter reusing A's indicators, node MLP, mean-pool partials;
     pool-reduce + MLP head at the end.
All matmul inputs bf16 (fp32 PSUM accumulate); head fp32. Streaming DMA
batched per super-tile and issued from the Pool-engine queue.
"""
import sys

sys.path.insert(0, "/opt/trn_rl_repo")

import math
from contextlib import ExitStack

import numpy as np

from concourse import bacc, bass, mybir, tile
from concourse import bass_utils
from concourse.masks import make_identity

F32 = mybir.dt.float32
BF16 = mybir.dt.bfloat16
I16 = mybir.dt.int16
AF = mybir.ActivationFunctionType
ALU = mybir.AluOpType
F32R = mybir.dt.float32r
NPBF = mybir.dt.np(BF16)
FR = lambda ap: ap.bitcast(F32R)

C = 8
P = 128
ND = 64
ED = 16
EMB = 128
USR = 12
KEXT = ED + 1 + ND     # [ea | 1 | x_src]
KE2 = ED + 1
G = 4096
NCH = 512
ETH = 256              # max edges per tile


def _wrap16(idx):
    L = len(idx)
    assert L % 16 == 0
    a = np.asarray(idx, np.int16).reshape(L // 16, 16).T
    return np.tile(a, (8, 1))


class CFG:
    pass


def _preprocess(x, edge_index, edge_attr, batch):
    x = np.asarray(x, np.float32)
    src = np.asarray(edge_index[0], np.int64)
    dst = np.asarray(edge_index[1], np.int64)
    batch = np.asarray(batch, np.int64)
    ea = np.asarray(edge_attr, np.float32)
    N = x.shape[0]

    GS = G // C
    gb = np.searchsorted(batch, np.arange(0, G + 1, GS))
    owner = np.searchsorted(gb, dst, side="right") - 1

    order_by_dst = np.argsort(dst, kind="stable")
    indeg = np.bincount(dst, minlength=N)
    in_start = np.concatenate([[0], np.cumsum(indeg)])

    # greedy variable tile boundaries per core: <=128 nodes, <=ETH edges
    cores = []
    for c in range(C):
        n_c = int(gb[c + 1] - gb[c])
        deg = indeg[gb[c]:gb[c + 1]]
        tile_of = np.zeros(n_c, np.int64)
        off_in = np.zeros(n_c, np.int64)
        t = nodes = edges = 0
        for v in range(n_c):
            if nodes >= P or edges + deg[v] > ETH:
                t += 1; nodes = 0; edges = 0
            tile_of[v] = t
            off_in[v] = nodes
            nodes += 1; edges += int(deg[v])
        cores.append(dict(n_c=n_c, tile_of=tile_of, off_in=off_in,
                          ntile=t + 1))

    NT = int(math.ceil(max(pc["ntile"] for pc in cores) / 4) * 4)
    N_SH = NT * P
    NST = NT // 4
    KA = ETH // P                      # blocks per tile (exactly 2)
    NBLK = NT * KA
    ESH_A = NBLK * P
    bo = np.arange(NT + 1) * KA

    # per-core edge -> slot
    for c, pc in enumerate(cores):
        em = np.nonzero(owner == c)[0]
        dloc = dst[em] - gb[c]
        t_of = pc["tile_of"][dloc]
        o = np.argsort(t_of, kind="stable")
        em, dloc, t_of = em[o], dloc[o], t_of[o]
        cnt_t = np.bincount(t_of, minlength=NT)
        assert cnt_t.max() <= ETH
        st_off = np.concatenate([[0], np.cumsum(cnt_t)])
        rank = np.arange(len(em)) - st_off[t_of]
        slot = t_of * ETH + rank
        pc.update(em=em, dloc=dloc, slot=slot)
        vsrc = np.full(ESH_A, -1, np.int64)
        vsrc[slot] = src[em]
        pc["vsrc"] = vsrc
        # node positions
        pos = pc["tile_of"] * P + pc["off_in"]
        pc["pos"] = pos

    # B group (= A/C block) edge counts
    kB = np.ones(NBLK, np.int64)
    for pc in cores:
        vs = pc["vsrc"]
        deg = np.where(vs >= 0, indeg[np.maximum(vs, 0)], 0)
        gcnt = deg.reshape(NBLK, P).sum(1)
        kB = np.maximum(kB, (gcnt + P - 1) // P)
    boB = np.concatenate([[0], np.cumsum(kB)])
    NBLKB = int(boB[-1])
    ESH_B = NBLKB * P

    cfg = CFG()
    cfg.N_SH, cfg.NT, cfg.NST, cfg.KA = N_SH, NT, NST, KA
    cfg.bo, cfg.NBLK, cfg.ESH_A = bo, NBLK, ESH_A
    cfg.kB, cfg.boB, cfg.NBLKB, cfg.ESH_B = kB, boB, NBLKB, ESH_B
    cfg.GS = GS
    cfg.GSP = max(P, int(math.ceil(GS / P) * P))
    assert kB.max() <= 8, kB.max()

    per_core = []
    for c, pc in enumerate(cores):
        em, slot, vsrc = pc["em"], pc["slot"], pc["vsrc"]
        n_c, pos = pc["n_c"], pc["pos"]
        eaExtA = np.zeros((KEXT, ESH_A), np.float32)
        eaExtA[:ED, slot] = ea[em].T
        eaExtA[ED, slot] = 1.0
        eaExtA[ED + 1:, slot] = x[src[em]].T
        dstrelA = np.full(ESH_A, 255.0, np.float32)
        dstrelA[slot] = pc["off_in"][pc["dloc"]].astype(np.float32)
        eaT2C = np.zeros((KE2, ESH_A), np.float32)
        eaT2C[:ED, slot] = ea[em].T
        eaT2C[ED, slot] = 1.0
        eaExtB = np.zeros((KEXT, ESH_B), np.float32)
        vrelB = np.full(ESH_B, 255.0, np.float32)
        xvT = np.zeros((ND, ESH_A), np.float32)
        real = vsrc >= 0
        xvT[:, real] = x[vsrc[real]].T
        deg = np.where(real, indeg[np.maximum(vsrc, 0)], 0)
        for g in range(NBLK):
            vv = vsrc[g * P:(g + 1) * P]
            dd = deg[g * P:(g + 1) * P]
            tot = int(dd.sum())
            if tot == 0:
                continue
            starts = in_start[np.maximum(vv, 0)]
            reps = np.repeat(starts, dd) + (
                np.arange(tot) - np.repeat(np.concatenate([[0], np.cumsum(dd)])[:-1], dd))
            eids = order_by_dst[reps]
            ppos = boB[g] * P + np.arange(tot)
            eaExtB[:ED, ppos] = ea[eids].T
            eaExtB[ED, ppos] = 1.0
            eaExtB[ED + 1:, ppos] = x[src[eids]].T
            vrelB[ppos] = np.repeat(np.arange(P), dd).astype(np.float32)

        # pooling structures on positions
        bl = batch[gb[c]:gb[c + 1]] - c * GS
        blp = np.full(N_SH, -1, np.int64)
        blp[pos] = bl
        tiles = blp.reshape(NT, P)
        g_first = np.array([t[t >= 0].min() if (t >= 0).any() else 0
                            for t in tiles])
        relg = np.where(blp >= 0, blp - np.repeat(g_first, P), 255.0)
        cnt = np.bincount(bl, minlength=GS).astype(np.float32)
        gstart = np.searchsorted(bl, np.arange(GS))
        gend = np.searchsorted(bl, np.arange(GS), side="right")
        t_lo = pc["tile_of"][np.minimum(gstart, n_c - 1)]
        t_hi = pc["tile_of"][np.maximum(gend - 1, gstart)]

        xT = np.zeros((ND, N_SH), np.float32)
        xT[:, pos] = x[gb[c]:gb[c + 1]].T

        per_core.append(dict(
            eaExtA=eaExtA, dstrelA=dstrelA.reshape(NBLK, P).T,
            eaT2C=eaT2C, eaExtB=eaExtB, vrelB=vrelB.reshape(NBLKB, P).T,
            xvT=xvT, n_c=n_c, xT=xT, pos=pos,
            relg=relg.astype(np.float32), g_first=g_first, cnt=cnt,
            t_lo=t_lo, t_hi=t_hi,
        ))

    cfg.GW = int(max((pc["relg"][pc["relg"] != 255.0]).max() + 1
                     if (pc["relg"] != 255.0).any() else 1 for pc in per_core))
    cfg.PG = int(max((pc["t_hi"] - pc["t_lo"] + 1)[pc["cnt"] > 0].max()
                     if (pc["cnt"] > 0).any() else 1 for pc in per_core))
    cfg.n_pool_idx = int(math.ceil(cfg.PG * cfg.GSP / P) * P)

    ZPAD = NT * cfg.GW
    for pc in per_core:
        pidx = np.full(cfg.n_pool_idx, ZPAD, np.int16)
        for g in range(GS):
            if pc["cnt"][g] <= 0:
                continue
            for p_, t in enumerate(range(pc["t_lo"][g], pc["t_hi"][g] + 1)):
                rel = g - pc["g_first"][t]
                pidx[p_ * cfg.GSP + g] = t * cfg.GW + rel
        pc["pool_idx"] = pidx
        pc["cnt_gm"] = np.maximum(
            np.pad(pc["cnt"], (0, cfg.GSP - GS)), 1.0
        ).reshape(cfg.GSP // P, P).T.astype(np.float32)

    relids = np.tile(np.arange(cfg.GW, dtype=np.float32), (P, 1))
    return cfg, gb, per_core, relids


def _build(cfg):
    nc = bacc.Bacc("TRN2", target_bir_lowering=False, debug=False,
                   num_devices=C)
    d = {}

    def inp(name, shape, dt=F32):
        d[name] = nc.dram_tensor(name, shape, dt, kind="ExternalInput").ap()

    NBLK, NBLKB, NT, NST, KA = cfg.NBLK, cfg.NBLKB, cfg.NT, cfg.NST, cfg.KA
    bo, kB, boB = cfg.bo, cfg.kB, cfg.boB
    GW, PG, GSP = cfg.GW, cfg.PG, cfg.GSP
    NROW = NT * GW + P

    inp("eaExtA", [KEXT, cfg.ESH_A], BF16)
    inp("eaExtB", [KEXT, cfg.ESH_B], BF16)
    inp("eaT2C", [KE2, cfg.ESH_A], BF16)
    inp("dstrelA", [P, NBLK], BF16)
    inp("vrelB", [P, NBLKB], BF16)
    inp("xvT", [ND, cfg.ESH_A])
    inp("xT", [ND, cfg.N_SH])
    inp("iota", [P, P], BF16)
    inp("W1e", [KEXT, ND], BF16)
    inp("W2eR", [ND + KE2, ND], BF16)
    inp("w11", [ND, ND], F32R)
    inp("b11", [ND, 1])
    inp("w12", [ND, ND], F32R)
    inp("b12", [ND, 1])
    inp("w21", [ND, EMB], F32R)
    inp("b21", [EMB, 1])
    inp("w22", [EMB, EMB], F32R)
    inp("b22", [EMB, 1])
    inp("relg", [P, NT], BF16)
    inp("relids", [P, GW], BF16)
    inp("pool_idx", [P, cfg.n_pool_idx // 16], I16)
    inp("cnt_gm", [P, GSP // P])
    inp("usrT", [USR, GSP])
    for nm, shp in (("hw1a", [EMB, 128]), ("hw1b", [USR, 128]), ("hb1", [128, 1]),
                    ("hw2", [128, 64]), ("hb2", [64, 1]), ("hw3", [64, 32]),
                    ("hb3", [32, 1]), ("hw4", [32, 16]), ("hb4", [16, 1]),
                    ("hw5", [16, 1]), ("hb5", [1, 1])):
        inp(nm, shp)
    yT = nc.dram_tensor("yT", [1, GSP], F32, kind="ExternalOutput").ap()

    with tile.TileContext(nc) as tc, ExitStack() as ctx:
        const = ctx.enter_context(tc.tile_pool(name="const", bufs=1))

        def ld(name, shape, dt=F32):
            t = const.tile(shape, dt, name=f"c_{name}")
            nc.sync.dma_start(t[:], d[name])
            return t

        W1e = ld("W1e", [KEXT, ND], BF16)
        W2eR = ld("W2eR", [ND + KE2, ND], BF16)
        w11 = ld("w11", [ND, ND], F32R); b11 = ld("b11", [ND, 1])
        w12 = ld("w12", [ND, ND], F32R); b12 = ld("b12", [ND, 1])
        w21 = ld("w21", [ND, EMB], F32R); b21 = ld("b21", [EMB, 1])
        w22 = ld("w22", [EMB, EMB], F32R); b22 = ld("b22", [EMB, 1])
        iota = ld("iota", [P, P], BF16)
        dstrelA = ld("dstrelA", [P, NBLK], BF16)
        vrelB = ld("vrelB", [P, NBLKB], BF16)
        relg = ld("relg", [P, NT], BF16)
        relids = ld("relids", [P, GW], BF16)
        ident = const.tile([P, P], F32, name="ident")
        make_identity(nc, ident[:])
        zt = const.tile([P, P], F32, name="zt")
        nc.vector.memset(zt[:], 0.0)

        dram = ctx.enter_context(tc.tile_pool(name="dram", bufs=1, space="DRAM"))
        parts = dram.tile([NROW, P], F32)
        nc.sync.dma_start(
            out=parts[NT * GW:NT * GW + P, :].rearrange("(p r) e -> p (r e)", p=P),
            in_=zt[:, :P])

        mctx = ctx.enter_context(ExitStack())
        psE = mctx.enter_context(tc.tile_pool(name="psE", bufs=3, space="PSUM"))
        psA = mctx.enter_context(tc.tile_pool(name="psA", bufs=3, space="PSUM"))
        psZ = mctx.enter_context(tc.tile_pool(name="psZ", bufs=2, space="PSUM"))
        eaAp = mctx.enter_context(tc.tile_pool(name="eaAp", bufs=2))
        eaBp = mctx.enter_context(tc.tile_pool(name="eaBp", bufs=2))
        ea2p = mctx.enter_context(tc.tile_pool(name="ea2p", bufs=2))
        xvp = mctx.enter_context(tc.tile_pool(name="xvp", bufs=2))
        msgp = mctx.enter_context(tc.tile_pool(name="msgp", bufs=6))
        indp = mctx.enter_context(tc.tile_pool(name="indp", bufs=16))
        hvp = mctx.enter_context(tc.tile_pool(name="hvp", bufs=4))
        z1p = mctx.enter_context(tc.tile_pool(name="z1p", bufs=4))
        hp = mctx.enter_context(tc.tile_pool(name="hp", bufs=3))
        h1p = mctx.enter_context(tc.tile_pool(name="h1p", bufs=2))
        embp = mctx.enter_context(tc.tile_pool(name="embp", bufs=2))
        s4p = mctx.enter_context(tc.tile_pool(name="s4p", bufs=2))
        psbp = mctx.enter_context(tc.tile_pool(name="psbp", bufs=2))
        xtp = mctx.enter_context(tc.tile_pool(name="xtp", bufs=2))

        for st in range(NST):
            t0, t1 = 4 * st, 4 * st + 4
            sA0, sA1 = bo[t0] * P, bo[t1] * P         # A/C slot range
            sB0, sB1 = boB[bo[t0]] * P, boB[bo[t1]] * P
            # ---- batched streaming loads for this super-tile ----
            eaA = eaAp.tile([KEXT, sA1 - sA0], BF16, tag="eaA")
            nc.gpsimd.dma_start(eaA[:], d["eaExtA"][:, sA0:sA1])
            eaB = eaBp.tile([KEXT, sB1 - sB0], BF16, tag="eaB")
            nc.gpsimd.dma_start(eaB[:], d["eaExtB"][:, sB0:sB1])
            e2R = ea2p.tile([ND + KE2, sA1 - sA0], BF16, tag="ea2")
            nc.gpsimd.dma_start(e2R[ND:, :], d["eaT2C"][:, sA0:sA1])
            xvt = xvp.tile([ND, sA1 - sA0], F32, tag="xvt")
            nc.gpsimd.dma_start(xvt[:], d["xvT"][:, sA0:sA1])
            xTs = xtp.tile([ND, NCH], F32, tag="xTs")
            nc.gpsimd.dma_start(xTs[:], d["xT"][:, st * NCH:(st + 1) * NCH])

            ind_of = {}
            # ---- B: recompute R rows, tile-pair chunks ----
            for pr in range(2):
                tp0 = t0 + 2 * pr
                L = KA * P
                L2 = 2 * L
                co2 = (bo[tp0] - bo[t0]) * P
                aggBs = []
                for t in (tp0, tp0 + 1):
                    aggB = psA.tile([ND, L], F32, tag="agg")
                    for gj in range(KA):
                        g = bo[t] + gj
                        kb = int(kB[g])
                        eL = psE.tile([P, kb * ND], F32, tag="eL")
                        msgB = msgp.tile([P, kb * ND], BF16, tag="msgB")
                        eoff = (boB[g] - boB[bo[t0]]) * P
                        for b in range(kb):
                            nc.tensor.matmul(
                                out=eL[:, b * ND:(b + 1) * ND],
                                lhsT=eaB[:, eoff + b * P:eoff + (b + 1) * P],
                                rhs=W1e[:], start=True, stop=True)
                        nc.scalar.activation(out=msgB[:], in_=eL[:], func=AF.Relu)
                        indB = indp.tile([P, kb * P], BF16, tag="indB")
                        nc.vector.tensor_tensor(
                            out=indB[:].rearrange("p (k e) -> p k e", e=P),
                            in0=vrelB[:, boB[g]:boB[g] + kb]
                            .unsqueeze(2).broadcast_to([P, kb, P]),
                            in1=iota[:].unsqueeze(1).broadcast_to([P, kb, P]),
                            op=ALU.is_equal)
                        for b in range(kb):
                            nc.tensor.matmul(out=aggB[:, gj * P:(gj + 1) * P],
                                             lhsT=msgB[:, b * ND:(b + 1) * ND],
                                             rhs=indB[:, b * P:(b + 1) * P],
                                             start=(b == 0), stop=(b == kb - 1))
                    aggBs.append(aggB)
                hv = hvp.tile([ND, L2], F32R, tag="hv")
                for i in range(2):
                    nc.vector.tensor_add(
                        out=hv[:, i * L:(i + 1) * L], in0=aggBs[i][:],
                        in1=xvt[:, co2 + i * L:co2 + (i + 1) * L])
                z1ps = psZ.tile([ND, L2], F32, tag="z")
                nc.tensor.matmul(out=z1ps[:], lhsT=w11[:], rhs=hv[:],
                                 start=True, stop=True)
                z1sb = z1p.tile([ND, L2], F32R, tag="z1B")
                nc.scalar.activation(out=z1sb[:], in_=z1ps[:], func=AF.Relu,
                                     bias=b11[:])
                Rps = psZ.tile([ND, L2], F32, tag="z")
                nc.tensor.matmul(out=Rps[:], lhsT=w12[:], rhs=z1sb[:],
                                 start=True, stop=True)
                nc.scalar.activation(out=e2R[:ND, co2:co2 + L2], in_=Rps[:],
                                     func=AF.Relu, bias=b12[:])
            # ---- A ----
            agg1 = psA.tile([ND, NCH], F32, tag="agg")
            for j, t in enumerate(range(t0, t1)):
                co = (bo[t] - bo[t0]) * P
                eL = psE.tile([P, KA * ND], F32, tag="eL")
                msgA = msgp.tile([P, KA * ND], BF16, tag="msgA")
                for b in range(KA):
                    nc.tensor.matmul(
                        out=eL[:, b * ND:(b + 1) * ND],
                        lhsT=eaA[:, co + b * P:co + (b + 1) * P],
                        rhs=W1e[:], start=True, stop=True)
                nc.scalar.activation(out=msgA[:], in_=eL[:], func=AF.Relu)
                indA = indp.tile([P, KA * P], BF16, tag="indA")
                nc.vector.tensor_tensor(
                    out=indA[:].rearrange("p (k e) -> p k e", e=P),
                    in0=dstrelA[:, bo[t]:bo[t] + KA]
                    .unsqueeze(2).broadcast_to([P, KA, P]),
                    in1=iota[:].unsqueeze(1).broadcast_to([P, KA, P]),
                    op=ALU.is_equal)
                ind_of[t] = indA
                for b in range(KA):
                    nc.tensor.matmul(out=agg1[:, j * P:(j + 1) * P],
                                     lhsT=msgA[:, b * ND:(b + 1) * ND],
                                     rhs=indA[:, b * P:(b + 1) * P],
                                     start=(b == 0), stop=(b == KA - 1))
            hT = hp.tile([ND, NCH], F32R, tag="hA")
            nc.vector.tensor_add(out=hT[:], in0=agg1[:], in1=xTs[:])
            z1ps = psZ.tile([ND, NCH], F32, tag="z")
            nc.tensor.matmul(out=z1ps[:], lhsT=w11[:], rhs=hT[:],
                             start=True, stop=True)
            z1sb = z1p.tile([ND, NCH], F32R, tag="z1A")
            nc.scalar.activation(out=z1sb[:], in_=z1ps[:], func=AF.Relu,
                                 bias=b11[:])
            h1ps = psZ.tile([ND, NCH], F32, tag="z")
            nc.tensor.matmul(out=h1ps[:], lhsT=w12[:], rhs=z1sb[:],
                             start=True, stop=True)
            h1T = h1p.tile([ND, NCH], F32, tag="h1T")
            nc.scalar.activation(out=h1T[:], in_=h1ps[:], func=AF.Relu,
                                 bias=b12[:])
            # ---- C ----
            agg2 = psA.tile([ND, NCH], F32, tag="agg")
            for j, t in enumerate(range(t0, t1)):
                co = (bo[t] - bo[t0]) * P
                eL2 = psE.tile([P, KA * ND], F32, tag="eL")
                for b in range(KA):
                    nc.tensor.matmul(
                        out=eL2[:, b * ND:(b + 1) * ND],
                        lhsT=e2R[:, co + b * P:co + (b + 1) * P],
                        rhs=W2eR[:], start=True, stop=True)
                msg2 = msgp.tile([P, KA * ND], BF16, tag="msg2")
                nc.scalar.activation(out=msg2[:], in_=eL2[:], func=AF.Relu)
                for b in range(KA):
                    nc.tensor.matmul(out=agg2[:, j * P:(j + 1) * P],
                                     lhsT=msg2[:, b * ND:(b + 1) * ND],
                                     rhs=ind_of[t][:, b * P:(b + 1) * P],
                                     start=(b == 0), stop=(b == KA - 1))
            hT2 = hp.tile([ND, NCH], F32R, tag="hC")
            nc.vector.tensor_add(out=hT2[:], in0=agg2[:], in1=h1T[:])
            z1ps2 = psZ.tile([EMB, NCH], F32, tag="z")
            nc.tensor.matmul(out=z1ps2[:], lhsT=w21[:], rhs=hT2[:],
                             start=True, stop=True)
            z1Csb = z1p.tile([EMB, NCH], F32R, tag="z1C")
            nc.scalar.activation(out=z1Csb[:], in_=z1ps2[:], func=AF.Relu,
                                 bias=b21[:])
            z2ps = psZ.tile([P, 4 * EMB], F32, tag="z")
            for j in range(4):
                nc.tensor.matmul(out=z2ps[:, j * EMB:(j + 1) * EMB],
                                 lhsT=z1Csb[:, j * P:(j + 1) * P],
                                 rhs=w22[:], start=True, stop=True)
            emb_nm = embp.tile([P, 4 * EMB], BF16, tag="emb")
            nc.scalar.activation(out=emb_nm[:], in_=z2ps[:], func=AF.Identity)
            S4 = s4p.tile([P, 4 * GW], BF16, tag="S4")
            nc.vector.tensor_tensor(
                out=S4[:].rearrange("p (k g) -> p k g", g=GW),
                in0=relg[:, t0:t1].unsqueeze(2).broadcast_to([P, 4, GW]),
                in1=relids[:].unsqueeze(1).broadcast_to([P, 4, GW]),
                op=ALU.is_equal)
            psb = psbp.tile([GW, 4 * P], F32, tag="psb")
            for j in range(4):
                pps = psZ.tile([GW, P], F32, tag="z")
                nc.tensor.matmul(out=pps[:], lhsT=S4[:, j * GW:(j + 1) * GW],
                                 rhs=emb_nm[:, j * EMB:(j + 1) * EMB],
                                 start=True, stop=True)
                nc.vector.tensor_copy(out=psb[:, j * P:(j + 1) * P], in_=pps[:])
            nc.gpsimd.dma_start(
                out=parts[t0 * GW:t1 * GW, :].rearrange("(k g) e -> g k e", g=GW),
                in_=psb[:].rearrange("g (k e) -> g k e", e=P))

        # ---- pool reduce + head ----
        mctx.close()
        with ExitStack() as hctx:
            hpool = hctx.enter_context(tc.tile_pool(name="hd", bufs=1))
            hps = hctx.enter_context(tc.tile_pool(name="hd_ps", bufs=2, space="PSUM"))
            pix = hpool.tile([P, cfg.n_pool_idx // 16], I16)
            nc.sync.dma_start(pix[:], d["pool_idx"])
            NPB = cfg.n_pool_idx // P
            gpo = hpool.tile([P, NPB * P], F32)
            nc.gpsimd.dma_gather(
                out_ap=gpo[:].rearrange("p (k e) -> p k e", e=P),
                in_ap=parts[:], idxs_ap=pix[:],
                num_idxs=cfg.n_pool_idx, num_idxs_reg=cfg.n_pool_idx,
                elem_size=P, single_packet=False)
            GB = GSP // P
            v = gpo[:].rearrange("p (q b e) -> p q b e", q=PG, b=GB)
            pooled = hpool.tile([P, GB * P], F32)
            pv = pooled[:].rearrange("p (b e) -> p b e", b=GB)
            if PG == 1:
                nc.vector.tensor_copy(out=pv, in_=v[:, 0])
            else:
                nc.vector.tensor_add(out=pv, in0=v[:, 0], in1=v[:, 1])
                for q in range(2, PG):
                    nc.vector.tensor_add(out=pv, in0=pv, in1=v[:, q])
            cntg = hpool.tile([P, GB], F32)
            nc.sync.dma_start(cntg[:], d["cnt_gm"])
            invc = hpool.tile([P, GB], F32)
            nc.vector.reciprocal(invc[:], cntg[:])
            for b in range(GB):
                nc.vector.tensor_tensor(
                    out=pooled[:, b * P:(b + 1) * P],
                    in0=pooled[:, b * P:(b + 1) * P],
                    in1=invc[:, b:b + 1].to_broadcast([P, P]), op=ALU.mult)
            embT = hpool.tile([P, GSP], F32)
            for b in range(GB):
                tps = hps.tile([P, P], F32, tag="hd")
                nc.tensor.transpose(out=tps[:], in_=pooled[:, b * P:(b + 1) * P],
                                    identity=ident[:])
                nc.vector.tensor_copy(out=embT[:, b * P:(b + 1) * P], in_=tps[:])
            nc.scalar.activation(out=embT[:], in_=embT[:], func=AF.Identity,
                                 bias=b22[:])
            usrT = hpool.tile([USR, GSP], F32)
            nc.sync.dma_start(usrT[:], d["usrT"])
            hw = {nm: hpool.tile(d[nm].shape, F32, name=f"t_{nm}")
                  for nm in ("hw1a", "hw1b", "hb1", "hw2", "hb2", "hw3", "hb3",
                             "hw4", "hb4", "hw5", "hb5")}
            for nm, t in hw.items():
                nc.sync.dma_start(t[:], d[nm])
            z1h = hps.tile([128, GSP], F32, tag="hd")
            nc.tensor.matmul(out=z1h[:], lhsT=hw["hw1a"][:], rhs=embT[:],
                             start=True, stop=False)
            nc.tensor.matmul(out=z1h[:], lhsT=hw["hw1b"][:], rhs=usrT[:],
                             start=False, stop=True)
            z1s = hpool.tile([128, GSP], F32)
            nc.scalar.activation(out=z1s[:], in_=z1h[:], func=AF.Relu, bias=hw["hb1"][:])
            z2h = hps.tile([64, GSP], F32, tag="hd")
            nc.tensor.matmul(out=z2h[:], lhsT=hw["hw2"][:], rhs=z1s[:], start=True, stop=True)
            z2s = hpool.tile([64, GSP], F32)
            nc.scalar.activation(out=z2s[:], in_=z2h[:], func=AF.Relu, bias=hw["hb2"][:])
            z3h = hps.tile([32, GSP], F32, tag="hd")
            nc.tensor.matmul(out=z3h[:], lhsT=hw["hw3"][:], rhs=z2s[:], start=True, stop=True)
            z3s = hpool.tile([32, GSP], F32)
            nc.scalar.activation(out=z3s[:], in_=z3h[:], func=AF.Relu, bias=hw["hb3"][:])
            z4h = hps.tile([16, GSP], F32, tag="hd")
            nc.tensor.matmul(out=z4h[:], lhsT=hw["hw4"][:], rhs=z3s[:], start=True, stop=True)
            z4s = hpool.tile([16, GSP], F32)
            nc.scalar.activation(out=z4s[:], in_=z4h[:], func=AF.Relu, bias=hw["hb4"][:])
            z5h = hps.tile([1, GSP], F32, tag="hd")
            nc.tensor.matmul(out=z5h[:], lhsT=hw["hw5"][:], rhs=z4s[:], start=True, stop=True)
            z5s = hpool.tile([1, GSP], F32)
            nc.scalar.activation(out=z5s[:], in_=z5h[:], func=AF.Identity, bias=hw["hb5"][:])
            nc.sync.dma_start(out=yT, in_=z5s[:])

    nc.compile()
    return nc


def _make_in_maps(cfg, gb, per_core, relids, inputs):
    f32 = lambda a: np.ascontiguousarray(np.asarray(a, np.float32))
    bf = lambda a: np.ascontiguousarray(np.asarray(a, np.float32)).astype(NPBF)
    W1ext = np.vstack([f32(inputs["e1_w"]), f32(inputs["e1_b"])[None, :],
                       np.eye(ND, dtype=np.float32)])
    W2ext = np.vstack([np.eye(ND, dtype=np.float32),
                       f32(inputs["e2_w"]), f32(inputs["e2_b"])[None, :]])
    usr = f32(inputs["usr"])
    iota = np.tile(np.arange(P, dtype=np.float32), (P, 1))
    in_maps = []
    for c, pc in enumerate(per_core):
        usrT = np.zeros((USR, cfg.GSP), np.float32)
        usrT[:, :cfg.GS] = usr[c * cfg.GS:(c + 1) * cfg.GS].T
        in_maps.append(dict(
            eaExtA=bf(pc["eaExtA"]), eaExtB=bf(pc["eaExtB"]),
            eaT2C=bf(pc["eaT2C"]), dstrelA=bf(pc["dstrelA"]),
            vrelB=bf(pc["vrelB"]), xvT=f32(pc["xvT"]),
            xT=f32(pc["xT"]), iota=bf(iota),
            W1e=bf(W1ext), W2eR=bf(W2ext),
            w11=f32(inputs["n1_w1"]), b11=f32(inputs["n1_b1"])[:, None],
            w12=f32(inputs["n1_w2"]), b12=f32(inputs["n1_b2"])[:, None],

            w21=f32(inputs["n2_w1"]), b21=f32(inputs["n2_b1"])[:, None],
            w22=f32(inputs["n2_w2"]), b22=f32(inputs["n2_b2"])[:, None],
            relg=bf(pc["relg"].reshape(cfg.NT, P).T), relids=bf(relids),
            pool_idx=_wrap16(pc["pool_idx"]), cnt_gm=pc["cnt_gm"], usrT=usrT,
            hw1a=f32(inputs["h1_w"])[:EMB], hw1b=f32(inputs["h1_w"])[EMB:],
            hb1=f32(inputs["h1_b"])[:, None],
            hw2=f32(inputs["h2_w"]), hb2=f32(inputs["h2_b"])[:, None],
            hw3=f32(inputs["h3_w"]), hb3=f32(inputs["h3_b"])[:, None],
            hw4=f32(inputs["h4_w"]), hb4=f32(inputs["h4_b"])[:, None],
            hw5=f32(inputs["h5_w"]), hb5=f32(inputs["h5_b"])[:, None]))
    return in_maps


def kernel(**inputs):
    cfg, gb, per_core, relids = _preprocess(
        np.asarray(inputs["x"], np.float32), inputs["edge_index"],
        np.asarray(inputs["edge_attr"], np.float32), inputs["batch"])
    nc = _build(cfg)
    in_maps = _make_in_maps(cfg, gb, per_core, relids, inputs)
    res = bass_utils.run_bass_kernel_spmd(nc, in_maps, core_ids=list(range(C)))
    out = np.concatenate([res.results[c]["yT"][0, :cfg.GS] for c in range(C)])
    kernel._last = res
    return out[:, None].astype(np.float32)
